# revision 1
# baseline (speedup 1.0000x reference)
"""Trainium2 Bass kernel: Llama-style attention block (prefill, start_pos=0).

Reference computation (per problem):
  q = x @ wq; k = x @ wk; v = x @ wv          (DIM=4096 -> 32 q-heads / 8 kv-heads, hd=128)
  rope(q, k) with interleaved (even, odd) pairs using freqs_cos/freqs_sin inputs
  scores = q @ k^T / sqrt(128) + mask ; p = softmax(scores) ; o = p @ v (GQA 4x)
  out = o @ wo

Distribution: tensor-parallel over heads on 8 cores. Core c owns q-heads
4c..4c+3 and kv-head c (GQA groups align with the core boundary), i.e.
wq/wk/wv are sharded column-wise and wo row-wise. Each core computes a
full-shape partial of the output projection; the host sums the 8 partials
(the row-parallel all-reduce, done on the host at unshard time).

Layout strategy on-chip (per core):
  - host passes x transposed (xT [4096, 2048]) so Q^T/K^T/V^T come out of the
    PE in dim-major layout [dims, seq], which is exactly the operand layout
    attention needs (contraction over head_dim = partition axis).
  - RoPE: wq/wk columns are permuted on the host so each head's rotation
    pairs (even, odd) become (first 64, last 64) rows. The pair swap
    [a;b] -> [-b;a] is then a constant 128x128 matmul on the PE, and the
    cos/sin combine is 3 elementwise DVE ops. Dot products are invariant
    under the permutation so scores match the reference exactly.
  - scores are computed transposed (S^T [k, q] blocks): softmax denominators
    become ones-vector matmuls on the PE (partition-axis reduction), exp
    runs on the scalar engine reading PSUM directly, and P^T feeds the
    P@V matmul with no transposes anywhere. V is transposed to seq-major
    once (16 PE transposes).
  - all matmul operands use float32r (E8M11): full PE rate at N>=256 with
    ~2.4e-4 element precision; PSUM accumulation stays fp32.
"""

import math

import numpy as np

import concourse.bass as bass
import concourse.mybir as mybir
import concourse.tile as tile
from concourse import bacc, bass_utils

DIM = 4096
N_HEADS = 32
N_KV = 8
HD = 128
SEQ = 2048
NCORES = 8
HPC = N_HEADS // NCORES          # q heads per core
QD = HPC * HD                    # 512 q-dims per core
SCALE = 1.0 / math.sqrt(HD)
NEG = -1.0e30

NQC = SEQ // 512                 # q chunks of 512
NKB = SEQ // 128                 # k blocks of 128
NKC = DIM // 128                 # contraction chunks of 128
XG = 2                           # kc chunks per x DMA group

F32 = mybir.dt.float32
F32R = mybir.dt.float32r
EXP = mybir.ActivationFunctionType.Exp

_PROG_CACHE = {}


def _build_program(mask_mode: str):
    """mask_mode: 'causal' (skip upper blocks, add triangular diagonal
    masks), 'none' (no masking), 'full' (add arbitrary maskT blocks)."""
    assert mask_mode in ("causal", "none", "full")
    nc = bacc.Bacc("TRN2", target_bir_lowering=False, debug=False,
                   num_devices=NCORES)

    xT = nc.dram_tensor("xT", [DIM, SEQ], F32R, kind="ExternalInput").ap()
    wq = nc.dram_tensor("wq", [DIM, QD], F32R, kind="ExternalInput").ap()
    wk = nc.dram_tensor("wk", [DIM, HD], F32R, kind="ExternalInput").ap()
    wv = nc.dram_tensor("wv", [DIM, HD], F32R, kind="ExternalInput").ap()
    wo = nc.dram_tensor("wo", [QD, DIM], F32R, kind="ExternalInput").ap()
    cos2 = nc.dram_tensor("cos2", [HD, SEQ], F32, kind="ExternalInput").ap()
    sin2 = nc.dram_tensor("sin2", [HD, SEQ], F32, kind="ExternalInput").ap()
    rmat = nc.dram_tensor("rmat", [HD, HD], F32R, kind="ExternalInput").ap()
    ident = nc.dram_tensor("ident", [128, 128], F32R, kind="ExternalInput").ap()
    ones_col_d = nc.dram_tensor("ones_col", [128, 1], F32R,
                                kind="ExternalInput").ap()
    ones_row_d = nc.dram_tensor("ones_row", [1, 128], F32R,
                                kind="ExternalInput").ap()
    if mask_mode == "causal":
        dmask_d = nc.dram_tensor("dmask", [4, 128, 512], F32,
                                 kind="ExternalInput").ap()
    if mask_mode == "full":
        maskT_d = nc.dram_tensor("maskT", [SEQ, SEQ], F32,
                                 kind="ExternalInput").ap()
    out = nc.dram_tensor("out", [SEQ, DIM], F32, kind="ExternalOutput").ap()

    with tile.TileContext(nc) as tc:
        with tc.tile_pool(name="persist", bufs=1) as pp:
            # ---- persistent tiles ----
            qt = [pp.tile([128, SEQ], F32R, name=f"qt{h}") for h in range(HPC)]
            kt = pp.tile([128, SEQ], F32R)
            vs = pp.tile([128, SEQ], F32R)        # seq-major V, block i at cols i*128
            rmat_sb = pp.tile([128, 128], F32R)
            ident_sb = pp.tile([128, 128], F32R)
            nc.sync.dma_start(ident_sb[:], ident[:])
            nc.sync.dma_start(rmat_sb[:], rmat[:])
            ones_sb = pp.tile([128, 1], F32R)
            nc.sync.dma_start(ones_sb[:], ones_col_d[:])
            onesrow = pp.tile([1, 128], F32R)
            nc.sync.dma_start(onesrow[:], ones_row_d[:])
            if mask_mode == "causal":
                dmask_sb = pp.tile([128, 4, 512], F32)
                nc.gpsimd.dma_start(dmask_sb[:],
                                    dmask_d.rearrange("r p q -> p r q"))

            # ================= Phase 1: QKV projections + RoPE =================
            psp = tc.alloc_tile_pool(name="ps", bufs=1, space="PSUM")
            # dummy matmuls on the identity tile keep the PE activity monitor
            # warm through the initial DMA window (else the first ~3.5us of
            # real matmuls run at half clock)
            warm = psp.tile([128, 128], F32, tag="vtr", bufs=1)
            for _ in range(50):
                nc.tensor.matmul(warm[:], ident_sb[:], ident_sb[:],
                                 start=True, stop=True)
            with tc.tile_pool(name="w1", bufs=1) as wp:
                # chunked weight loads so the first matmuls start after ~2MB
                wq_sb = wp.tile([128, NKC, QD], F32R)
                wk_sb = wp.tile([128, NKC, HD], F32R)
                wv_sb = wp.tile([128, NKC, HD], F32R)
                for k0, k1 in ((0, 2), (2, 8), (8, 20), (20, 32)):
                    ksl = slice(k0, k1)
                    for wsb, wdr in ((wq_sb, wq), (wk_sb, wk), (wv_sb, wv)):
                        w3 = wdr.rearrange("(kc p) m -> p kc m", p=128)
                        nc.sync.dma_start(wsb[:, ksl, :], w3[:, ksl, :])
                cos_sb = wp.tile([128, SEQ], F32)
                sin_sb = wp.tile([128, SEQ], F32)

                xT3 = xT.rearrange("(kc p) s -> kc p s", p=128)

                def drain_copy(m, pt, n):
                    # pass A: free the accumulator bank and launch the PE part
                    # (swap matmul / transposes) with nothing else in between,
                    # so neither the DVE nor the PE stream head-of-line blocks
                    # on rope arithmetic of an earlier head
                    raw = wp.tile([128, 512], F32R, tag="raw", bufs=6,
                                  name=f"raw{n}_{m}")
                    if m % 2 == 0:
                        nc.scalar.copy(raw[:], pt[:])
                    else:
                        nc.vector.tensor_copy(raw[:], pt[:])
                    if m <= HPC:
                        swp = psp.tile([128, 512], F32,
                                       tag=("aux" if m % 2 == 0 else "vtr"),
                                       bufs=1, name=f"swp{n}_{m}")
                        nc.tensor.matmul(swp[:], rmat_sb[:], raw[:],
                                         start=True, stop=True)
                        return raw, swp
                    pvts = []
                    for b in range(4):
                        pvt = psp.tile([128, 128], F32R, tag="vtr", bufs=1,
                                       name=f"pvt{n}_{b}")
                        nc.tensor.transpose(pvt[:], raw[:, b * 128:(b + 1) * 128],
                                            ident_sb[:])
                        pvts.append(pvt)
                    return raw, pvts

                def drain_rope_a(m, raw, pe_out, n, nsl):
                    # pass B1: drain the swap/transpose PSUM banks straight
                    # into the destination (partial rope: dst = swp*sin)
                    if m <= HPC:
                        dst = qt[m] if m < HPC else kt
                        nc.vector.tensor_mul(dst[:, nsl], pe_out[:],
                                             sin_sb[:, nsl])
                        return None
                    for b, pvt in enumerate(pe_out):
                        i = n * 4 + b
                        nc.vector.tensor_copy(vs[:, i * 128:(i + 1) * 128],
                                              pvt[:])
                    return None

                def drain_rope_b(m, raw, tmp, n, nsl):
                    # pass B2: finish the rope combine (dst += raw*cos)
                    if m <= HPC:
                        dst = qt[m] if m < HPC else kt
                        tmp2 = wp.tile([128, 512], F32, tag="ropetmp", bufs=2,
                                       name=f"tmp{n}_{m}")
                        nc.vector.tensor_mul(tmp2[:], raw[:], cos_sb[:, nsl])
                        nc.vector.tensor_add(dst[:, nsl], dst[:, nsl], tmp2[:])

                for n in range(NQC):
                    nsl = slice(n * 512, (n + 1) * 512)
                    waves = [list(range(HPC + 2))]
                    for w_i, mlist in enumerate(waves):
                        pts = {m: psp.tile([128, 512], F32, tag="big", bufs=6,
                                           name=f"pt{n}_{w_i}_{m}")
                               for m in mlist}
                        for g in range(NKC // XG):
                            xg = wp.tile([128, XG, 512], F32R, tag="xg", bufs=5)
                            nc.scalar.dma_start(
                                xg[:], xT3[g * XG:(g + 1) * XG, :, nsl]
                                .rearrange("kc p s -> p kc s"))
                            for kk in range(XG):
                                kc = g * XG + kk
                                st, sp = (kc == 0), (kc == NKC - 1)
                                for m in mlist:
                                    if m < HPC:
                                        w_ap = wq_sb[:, kc, m * 128:(m + 1) * 128]
                                    elif m == HPC:
                                        w_ap = wk_sb[:, kc, :]
                                    else:
                                        w_ap = wv_sb[:, kc, :]
                                    nc.tensor.matmul(pts[m][:], w_ap,
                                                     xg[:, kk, :],
                                                     start=st, stop=sp)
                        if n == 0 and w_i == 0:
                            # cos/sin are first needed here; deferring their
                            # 2MB load keeps early HBM bandwidth for x/weights
                            nc.sync.dma_start(cos_sb[:], cos2[:])
                            nc.sync.dma_start(sin_sb[:], sin2[:])
                        handles = {m: drain_copy(m, pts[m], n) for m in mlist}
                        tmps = {m: drain_rope_a(m, *handles[m], n, nsl)
                                for m in mlist}
                        for m in mlist:
                            drain_rope_b(m, handles[m][0], tmps[m], n, nsl)

            # ========== Phase 2+3: attention interleaved with out-proj ==========
            # One merged stream: after attention finishes q-chunk j, the
            # output projection for seq blocks 4j..4j+3 runs while chunk j+1's
            # attention pipeline fills — keeps the PE dense (HAM stays warm).
            with tc.tile_pool(name="pp2", bufs=1) as pp2:
                attn = [pp2.tile([128, SEQ], F32R, name=f"attn{h}")
                        for h in range(HPC)]

                with tc.tile_pool(name="att", bufs=1) as ap_:
                    wo_sb = ap_.tile([128, HPC, DIM], F32R)
                    wo3 = wo.rearrange("(kc p) n -> p kc n", p=128)
                    for g in range(2):
                        nc.sync.dma_start(wo_sb[:, :, g * 2048:(g + 1) * 2048],
                                          wo3[:, :, g * 2048:(g + 1) * 2048])

                    def wo_block(m):
                        # one 128-row seq block of the output projection
                        msl = slice(m * 128, (m + 1) * 128)
                        for w4 in range(4):
                            yps = [psp.tile([128, 512], F32, tag="big", bufs=6,
                                            name=f"yp{m}_{w4}_{i}")
                                   for i in range(2)]
                            for kc in range(HPC):
                                for i in range(2):
                                    ncol = w4 * 2 + i
                                    nc.tensor.matmul(
                                        yps[i][:], attn[kc][:, msl],
                                        wo_sb[:, kc, ncol * 512:(ncol + 1) * 512],
                                        start=(kc == 0), stop=(kc == HPC - 1))
                            for i in range(2):
                                ncol = w4 * 2 + i
                                ysb = ap_.tile([128, 512], F32, tag="ysb",
                                               bufs=6)
                                nc.vector.tensor_copy(ysb[:], yps[i][:])
                                nc.sync.dma_start(
                                    out[msl, ncol * 512:(ncol + 1) * 512],
                                    ysb[:])

                    for j in range(NQC):
                        jsl = slice(j * 512, (j + 1) * 512)
                        nblk = 4 * j + 4 if mask_mode == "causal" else NKB
                        for h in range(HPC):
                            # previous chunk's out-proj traced ahead of this
                            # h-chain: dense PE filler under the softmax chain
                            if j > 0:
                                wo_block(4 * (j - 1) + h)
                            dn = psp.tile([1, 512], F32, tag="aux", bufs=1,
                                          name=f"dn{h}_{j}")
                            pv = psp.tile([128, 512], F32, tag="big", bufs=6,
                                          name=f"pv{h}_{j}")
                            for i in range(nblk):
                                r = i - 4 * j
                                off = 128 * r if (mask_mode == "causal" and r > 0) else 0
                                qof = j * 512 + off
                                stp = psp.tile([128, 512], F32, tag="big", bufs=6,
                                               name=f"st{h}_{j}_{i}")
                                nc.tensor.matmul(stp[:, off:],
                                                 kt[:, i * 128:(i + 1) * 128],
                                                 qt[h][:, qof:(j + 1) * 512],
                                                 start=True, stop=True)
                                if mask_mode == "causal" and r >= 0:
                                    nc.vector.tensor_add(
                                        stp[:, off:], stp[:, off:],
                                        dmask_sb[:, r, off:])
                                elif mask_mode == "full":
                                    mt = ap_.tile([128, 512], F32, tag="mt", bufs=3)
                                    nc.sync.dma_start(
                                        mt[:], maskT_d[i * 128:(i + 1) * 128, jsl])
                                    nc.vector.tensor_add(stp[:], stp[:], mt[:])
                                pexp = ap_.tile([128, 512], F32R, tag="pexp",
                                                bufs=6, name=f"pexp{h}_{j}_{i}")
                                nc.scalar.activation(pexp[:, off:], stp[:, off:],
                                                     EXP, scale=SCALE)
                                nc.tensor.matmul(dn[:, off:], ones_sb[:],
                                                 pexp[:, off:],
                                                 start=(i == 0),
                                                 stop=(i == nblk - 1))
                                nc.tensor.matmul(pv[:, off:],
                                                 vs[:, i * 128:(i + 1) * 128],
                                                 pexp[:, off:],
                                                 start=(i == 0),
                                                 stop=(i == nblk - 1))
                            rcp = ap_.tile([1, 512], F32, tag="rcp", bufs=2)
                            nc.vector.reciprocal_approx_fast(rcp[:], dn[:])
                            rcpr = ap_.tile([1, 512], F32R, tag="rcpr", bufs=2)
                            nc.vector.tensor_copy(rcpr[:], rcp[:])
                            bc = psp.tile([128, 512], F32, tag="vtr", bufs=1,
                                          name=f"bc{h}_{j}")
                            nc.tensor.matmul(bc[:], onesrow[:], rcpr[:],
                                             start=True, stop=True)
                            bcs = ap_.tile([128, 512], F32, tag="bcs", bufs=2)
                            nc.vector.tensor_copy(bcs[:], bc[:])
                            nc.vector.tensor_mul(attn[h][:, jsl], pv[:], bcs[:])

                    for m in range(4 * (NQC - 1), 4 * NQC):
                        wo_block(m)
            psp.release()

    nc.compile()
    return nc


def get_program(mask_mode: str):
    if mask_mode not in _PROG_CACHE:
        _PROG_CACHE[mask_mode] = _build_program(mask_mode)
    return _PROG_CACHE[mask_mode]


# ====================== host-side preparation ======================

_PERM128 = np.concatenate([np.arange(0, 128, 2), np.arange(1, 128, 2)])


def _perm_cols(w: np.ndarray, n_heads: int) -> np.ndarray:
    """Permute each head's 128 columns: even dims first, odd dims last."""
    cols = np.concatenate([h * 128 + _PERM128 for h in range(n_heads)])
    return w[:, cols]


def _classify_mask(mask: np.ndarray) -> str:
    if not np.any(mask):
        return "none"
    iu = np.triu_indices(SEQ, 1)
    upper = mask[iu]
    lower_ok = not np.any(np.tril(mask))
    upper_ok = bool(np.all(np.isneginf(upper) | (upper <= -1e9)))
    if lower_ok and upper_ok:
        return "causal"
    return "full"


def _host_inputs(x, wq, wk, wv, wo, freqs_cos, freqs_sin, mask):
    x2 = np.ascontiguousarray(x.reshape(SEQ, DIM).T)        # xT [DIM, SEQ]
    wq_p = _perm_cols(np.asarray(wq, np.float32), N_HEADS)
    wk_p = _perm_cols(np.asarray(wk, np.float32), N_KV)
    wv_ = np.asarray(wv, np.float32)
    wo_ = np.asarray(wo, np.float32)

    cosT = np.asarray(freqs_cos, np.float32).T              # [64, SEQ]
    sinT = np.asarray(freqs_sin, np.float32).T
    cos2 = np.ascontiguousarray(np.concatenate([cosT, cosT], 0))  # [128, SEQ]
    sin2 = np.ascontiguousarray(np.concatenate([sinT, sinT], 0))

    rmat = np.zeros((HD, HD), np.float32)
    rmat[np.arange(64) + 64, np.arange(64)] = -1.0   # swp[:64] = -raw[64:]
    rmat[np.arange(64), np.arange(64) + 64] = 1.0    # swp[64:] = raw[:64]
    ident = np.eye(128, dtype=np.float32)

    mask = np.asarray(mask, np.float32)
    mode = _classify_mask(mask)

    common = {"xT": x2, "cos2": cos2, "sin2": sin2, "rmat": rmat,
              "ident": ident,
              "ones_col": np.ones((HD, 1), np.float32),
              "ones_row": np.ones((1, HD), np.float32)}
    if mode == "causal":
        kk = np.arange(128)[:, None]
        qq = np.arange(512)[None, :]
        dmask = np.stack([
            np.where(kk <= qq - 128 * r, 0.0, NEG).astype(np.float32)
            for r in range(4)])
        common["dmask"] = dmask
    elif mode == "full":
        m = np.where(np.isneginf(mask), NEG, mask)
        common["maskT"] = np.ascontiguousarray(m.T)

    in_maps = []
    for c in range(NCORES):
        im = dict(common)
        im["wq"] = np.ascontiguousarray(wq_p[:, c * QD:(c + 1) * QD])
        im["wk"] = np.ascontiguousarray(wk_p[:, c * HD:(c + 1) * HD])
        im["wv"] = np.ascontiguousarray(wv_[:, c * HD:(c + 1) * HD])
        im["wo"] = np.ascontiguousarray(wo_[c * QD:(c + 1) * QD, :])
        in_maps.append(im)
    return mode, in_maps


def _scores_safe(x, wq, wk):
    """The device softmax skips the max-subtraction (scores from
    setup_inputs()-scaled weights are O(5), so exp() is exact and safe).
    Estimate the score magnitude; if exp could overflow fp32, fall back."""
    sx = float(np.sqrt(np.mean(np.square(x), dtype=np.float64)))
    sq = sx * float(np.sqrt(np.mean(np.square(wq), dtype=np.float64)) * np.sqrt(DIM))
    sk = sx * float(np.sqrt(np.mean(np.square(wk), dtype=np.float64)) * np.sqrt(DIM))
    # rope with arbitrary freqs can scale q/k by ~sqrt(2); 7 sigma tail margin
    return 2.0 * sq * sk * 7.0 < 80.0


def _numpy_fallback(x, wq, wk, wv, wo, freqs_cos, freqs_sin, mask):
    """Slow but numerically-safe host path (stable softmax), used only when
    the score magnitudes could overflow the device's unshifted exp."""
    x2 = x.reshape(SEQ, DIM).astype(np.float64)
    q = (x2 @ wq.astype(np.float64)).reshape(SEQ, N_HEADS, HD)
    k = (x2 @ wk.astype(np.float64)).reshape(SEQ, N_KV, HD)
    v = (x2 @ wv.astype(np.float64)).reshape(SEQ, N_KV, HD)
    cos = freqs_cos.astype(np.float64)[:, None, :]
    sin = freqs_sin.astype(np.float64)[:, None, :]

    def rope(t):
        a, b = t[..., 0::2], t[..., 1::2]
        out = np.empty_like(t)
        out[..., 0::2] = a * cos - b * sin
        out[..., 1::2] = a * sin + b * cos
        return out

    q, k = rope(q), rope(k)
    m64 = mask.astype(np.float64)
    outh = np.empty((SEQ, N_HEADS, HD))
    for h in range(N_HEADS):
        g = h // (N_HEADS // N_KV)
        s = q[:, h, :] @ k[:, g, :].T / math.sqrt(HD) + m64
        p = np.exp(s - s.max(-1, keepdims=True))
        p /= p.sum(-1, keepdims=True)
        outh[:, h, :] = p @ v[:, g, :]
    y = outh.reshape(SEQ, N_HEADS * HD) @ wo.astype(np.float64)
    return y.astype(np.float32).reshape(1, SEQ, DIM)


def kernel(x, wq, wk, wv, wo, freqs_cos, freqs_sin, mask, cache_k, cache_v,
           start_pos, **_unused):
    sp = int(np.asarray(start_pos))
    x = np.asarray(x, np.float32)
    wq = np.asarray(wq, np.float32)
    wk = np.asarray(wk, np.float32)
    wv = np.asarray(wv, np.float32)
    wo = np.asarray(wo, np.float32)
    mask = np.asarray(mask, np.float32)
    if sp != 0:
        raise NotImplementedError("kernel assumes start_pos == 0 prefill")
    if not _scores_safe(x, wq, wk):
        return _numpy_fallback(x, wq, wk, wv, wo,
                               np.asarray(freqs_cos, np.float32),
                               np.asarray(freqs_sin, np.float32), mask)

    mode, in_maps = _host_inputs(x, wq, wk, wv, wo,
                                 freqs_cos, freqs_sin, mask)
    nc = get_program(mode)
    res = bass_utils.run_bass_kernel_spmd(nc, in_maps,
                                          core_ids=list(range(NCORES)))
    acc = np.zeros((SEQ, DIM), np.float64)
    for r in res.results:
        acc += r["out"].astype(np.float64)
    return acc.astype(np.float32).reshape(1, SEQ, DIM)



# revision 5
# speedup vs baseline: 1.0783x; 1.0783x over previous
"""Trainium2 Bass kernel: Llama-style attention block (prefill, start_pos=0).

Reference computation (per problem):
  q = x @ wq; k = x @ wk; v = x @ wv          (DIM=4096 -> 32 q-heads / 8 kv-heads, hd=128)
  rope(q, k) with interleaved (even, odd) pairs using freqs_cos/freqs_sin inputs
  scores = q @ k^T / sqrt(128) + mask ; p = softmax(scores) ; o = p @ v (GQA 4x)
  out = o @ wo

Distribution: tensor-parallel over heads on 8 cores. Core c owns q-heads
4c..4c+3 and kv-head c (GQA groups align with the core boundary), i.e.
wq/wk/wv are sharded column-wise and wo row-wise. Each core computes a
full-shape partial of the output projection; the host sums the 8 partials
(the row-parallel all-reduce, done on the host at unshard time).

Layout strategy on-chip (per core):
  - host passes x transposed (xT [4096, 2048]) so Q^T/K^T/V^T come out of the
    PE in dim-major layout [dims, seq], which is exactly the operand layout
    attention needs (contraction over head_dim = partition axis).
  - RoPE: wq/wk columns are permuted on the host so each head's rotation
    pairs (even, odd) become (first 64, last 64) rows. The pair swap
    [a;b] -> [-b;a] is then a constant 128x128 matmul on the PE, and the
    cos/sin combine is 3 elementwise DVE ops. Dot products are invariant
    under the permutation so scores match the reference exactly.
  - scores are computed transposed (S^T [k, q] blocks): exp runs on the
    scalar engine reading PSUM directly, and P^T feeds the P@V matmul with
    no transposes anywhere. V is transposed to seq-major once.
  - everything is bf16 (PSUM accumulation stays fp32): halves HBM traffic
    and doubles DVE elementwise throughput vs fp32; measured end-to-end
    error ~4e-3 vs the 2e-2 gate.
  - softmax denominators: pairs of pexp k-blocks are summed on the DVE
    (single bf16 rounding each) and contracted with a ones-vector matmul
    per pair; diagonal (causally-partial) blocks get their own ones-matmul.
    The causal mask inside a diagonal block is a [128,128] 0/1 bf16
    multiply on the DVE (upper blocks are skipped entirely).
  - pipeline: pass 1 computes K^T/V^T and q-head 0 for all chunks (3 PSUM
    accumulation banks); pass 2 per chunk j emits the q1..q3 projections
    interleaved with attention for chunk j-1 and the output projection for
    chunk j-2, so the PE never drains around the softmax latency chains.
"""

import math

import numpy as np

import concourse.bass as bass
import concourse.mybir as mybir
import concourse.tile as tile
from concourse import bacc, bass_utils

DIM = 4096
N_HEADS = 32
N_KV = 8
HD = 128
SEQ = 2048
NCORES = 8
HPC = N_HEADS // NCORES          # q heads per core
QD = HPC * HD                    # 512 q-dims per core
SCALE = 1.0 / math.sqrt(HD)
NEG = -1.0e30

NQC = SEQ // 512                 # q chunks of 512
NKB = SEQ // 128                 # k blocks of 128
NKC = DIM // 128                 # contraction chunks of 128
XG = 2                           # kc chunks per x DMA group

F32 = mybir.dt.float32
BF = mybir.dt.bfloat16
EXP = mybir.ActivationFunctionType.Exp

_PROG_CACHE = {}


def _build_program(mask_mode: str):
    """mask_mode: 'causal' (skip upper blocks, multiplicative triangular
    diagonal mask), 'none' (no masking), 'full' (add arbitrary maskT)."""
    assert mask_mode in ("causal", "none", "full")
    nc = bacc.Bacc("TRN2", target_bir_lowering=False, debug=False,
                   num_devices=NCORES)

    xT = nc.dram_tensor("xT", [DIM, SEQ], BF, kind="ExternalInput").ap()
    wq = nc.dram_tensor("wq", [DIM, QD], BF, kind="ExternalInput").ap()
    wk = nc.dram_tensor("wk", [DIM, HD], BF, kind="ExternalInput").ap()
    wv = nc.dram_tensor("wv", [DIM, HD], BF, kind="ExternalInput").ap()
    wo = nc.dram_tensor("wo", [QD, DIM], BF, kind="ExternalInput").ap()
    cos2 = nc.dram_tensor("cos2", [HD, SEQ], BF, kind="ExternalInput").ap()
    sin2 = nc.dram_tensor("sin2", [HD, SEQ], BF, kind="ExternalInput").ap()
    rmat = nc.dram_tensor("rmat", [HD, HD], BF, kind="ExternalInput").ap()
    ident = nc.dram_tensor("ident", [128, 128], BF, kind="ExternalInput").ap()
    ones_col_d = nc.dram_tensor("ones_col", [128, 1], BF,
                                kind="ExternalInput").ap()
    ones_row_d = nc.dram_tensor("ones_row", [1, 128], BF,
                                kind="ExternalInput").ap()
    if mask_mode == "causal":
        trimask_d = nc.dram_tensor("trimask", [128, 128], BF,
                                   kind="ExternalInput").ap()
    if mask_mode == "full":
        maskT_d = nc.dram_tensor("maskT", [SEQ, SEQ], F32,
                                 kind="ExternalInput").ap()
    out = nc.dram_tensor("out", [SEQ, DIM], BF, kind="ExternalOutput").ap()

    with tile.TileContext(nc) as tc:
        with tc.tile_pool(name="persist", bufs=1) as pp:
            # ---- persistent tiles ----
            qt = [pp.tile([128, SEQ], BF, name=f"qt{h}") for h in range(HPC)]
            kt = pp.tile([128, SEQ], BF)
            vs = pp.tile([128, SEQ], BF)         # seq-major V, block i at cols i*128
            attn = [pp.tile([128, SEQ], BF, name=f"attn{h}")
                    for h in range(HPC)]
            rmat_sb = pp.tile([128, 128], BF)
            ident_sb = pp.tile([128, 128], BF)
            ones_sb = pp.tile([128, 1], BF)
            onesrow = pp.tile([1, 128], BF)
            nc.gpsimd.dma_start(ident_sb[:], ident[:])
            nc.gpsimd.dma_start(rmat_sb[:], rmat[:])
            nc.gpsimd.dma_start(ones_sb[:], ones_col_d[:])
            nc.gpsimd.dma_start(onesrow[:], ones_row_d[:])
            if mask_mode == "causal":
                trimask_sb = pp.tile([128, 128], BF)
                nc.gpsimd.dma_start(trimask_sb[:], trimask_d[:])
            cos_sb = pp.tile([128, SEQ], BF)
            sin_sb = pp.tile([128, SEQ], BF)
            wq_sb = pp.tile([128, NKC, QD], BF)
            wk_sb = pp.tile([128, NKC, HD], BF)
            wv_sb = pp.tile([128, NKC, HD], BF)
            wo_sb = pp.tile([128, HPC, DIM], BF)

            # pass-1 weight loads (wk, wv, wq head0) then cos/sin, then the
            # rest of wq. wo is deferred into pass 2 (first needed ~130us in)
            # to keep early HBM bandwidth for x.
            wk3 = wk.rearrange("(kc p) m -> p kc m", p=128)
            wv3 = wv.rearrange("(kc p) m -> p kc m", p=128)
            wq3 = wq.rearrange("(kc p) m -> p kc m", p=128)
            nc.sync.dma_start(wk_sb[:, 0:8, :], wk3[:, 0:8, :])
            nc.sync.dma_start(wv_sb[:, 0:8, :], wv3[:, 0:8, :])
            nc.sync.dma_start(wq_sb[:, 0:8, 0:128], wq3[:, 0:8, 0:128])
            nc.sync.dma_start(wk_sb[:, 8:32, :], wk3[:, 8:32, :])
            nc.sync.dma_start(wv_sb[:, 8:32, :], wv3[:, 8:32, :])
            nc.sync.dma_start(wq_sb[:, 8:32, 0:128], wq3[:, 8:32, 0:128])
            nc.sync.dma_start(cos_sb[:], cos2[:])
            nc.sync.dma_start(sin_sb[:], sin2[:])
            nc.sync.dma_start(wq_sb[:, :, 128:512], wq3[:, :, 128:512])

            xT3 = xT.rearrange("(kc p) s -> kc p s", p=128)
            wo3 = wo.rearrange("(kc p) n -> p kc n", p=128)

            # ================= pass 1: K, V, q-head0 =================
            ps1 = tc.alloc_tile_pool(name="ps1", bufs=1, space="PSUM")
            # dummy matmuls on the identity tile keep the PE activity monitor
            # warm through the initial DMA window (else the first ~3.5us of
            # real matmuls run at half clock)
            warm = ps1.tile([128, 128], F32, tag="aux", bufs=2)
            for _ in range(48):
                nc.tensor.matmul(warm[:], ident_sb[:], ident_sb[:],
                                 start=True, stop=True)

            with tc.tile_pool(name="work", bufs=1) as wp:

                def rope_drain(head_or_k, n, acc):
                    # acc [128,512] f32 PSUM -> rope -> qt[h]/kt bf16
                    nsl = slice(n * 512, (n + 1) * 512)
                    dst = kt if head_or_k == "k" else qt[head_or_k]
                    raw = wp.tile([128, 512], BF, tag="raw", bufs=4,
                                  name=f"raw{head_or_k}_{n}")
                    nc.scalar.copy(raw[:], acc[:])
                    swp = ps1.tile([128, 512], F32, tag="aux", bufs=2,
                                   name=f"swp{head_or_k}_{n}")
                    nc.tensor.matmul(swp[:], rmat_sb[:], raw[:],
                                     start=True, stop=True)
                    nc.vector.tensor_mul(dst[:, nsl], swp[:], sin_sb[:, nsl])
                    tmp = wp.tile([128, 512], BF, tag="ropetmp", bufs=2,
                                  name=f"tmp{head_or_k}_{n}")
                    nc.vector.tensor_mul(tmp[:], raw[:], cos_sb[:, nsl])
                    nc.vector.tensor_add(dst[:, nsl], dst[:, nsl], tmp[:])

                def rope_drain2(head_or_k, n, acc, psp):
                    # same but allocating the swap tile from the pass-2 pool
                    nsl = slice(n * 512, (n + 1) * 512)
                    dst = kt if head_or_k == "k" else qt[head_or_k]
                    raw = wp.tile([128, 512], BF, tag="raw", bufs=4,
                                  name=f"raw{head_or_k}_{n}")
                    nc.scalar.copy(raw[:], acc[:])
                    swp = psp.tile([128, 512], F32, tag="big", bufs=3,
                                   name=f"swp{head_or_k}_{n}")
                    nc.tensor.matmul(swp[:], rmat_sb[:], raw[:],
                                     start=True, stop=True)
                    nc.vector.tensor_mul(dst[:, nsl], swp[:], sin_sb[:, nsl])
                    tmp = wp.tile([128, 512], BF, tag="ropetmp", bufs=2,
                                  name=f"tmp{head_or_k}_{n}")
                    nc.vector.tensor_mul(tmp[:], raw[:], cos_sb[:, nsl])
                    nc.vector.tensor_add(dst[:, nsl], dst[:, nsl], tmp[:])

                def v_drain(n, acc):
                    raw = wp.tile([128, 512], BF, tag="raw", bufs=4,
                                  name=f"rawv_{n}")
                    nc.vector.tensor_copy(raw[:], acc[:])
                    vtr = ps1.tile([128, 512], BF, tag="aux", bufs=2,
                                   name=f"vtr_{n}")
                    for b in range(4):
                        nc.tensor.transpose(vtr[:, b * 128:(b + 1) * 128],
                                            raw[:, b * 128:(b + 1) * 128],
                                            ident_sb[:])
                    nc.scalar.copy(vs[:, n * 512:(n + 1) * 512], vtr[:])

                # ---- pass 1 main loop ----
                for n in range(NQC):
                    nsl = slice(n * 512, (n + 1) * 512)
                    acc_k = ps1.tile([128, 512], F32, tag="acc", bufs=4,
                                     name=f"acck_{n}")
                    acc_v = ps1.tile([128, 512], F32, tag="acc", bufs=4,
                                     name=f"accv_{n}")
                    acc_q = ps1.tile([128, 512], F32, tag="acc", bufs=4,
                                     name=f"accq_{n}")
                    for g in range(NKC // XG):
                        xg = wp.tile([128, XG, 512], BF, tag="xg", bufs=4,
                                     name=f"xg1_{n}_{g}")
                        nc.scalar.dma_start(
                            xg[:], xT3[g * XG:(g + 1) * XG, :, nsl]
                            .rearrange("kc p s -> p kc s"))
                        for kk in range(XG):
                            kc = g * XG + kk
                            st, sp = (kc == 0), (kc == NKC - 1)
                            nc.tensor.matmul(acc_k[:], wk_sb[:, kc, :],
                                             xg[:, kk, :], start=st, stop=sp)
                            nc.tensor.matmul(acc_v[:], wv_sb[:, kc, :],
                                             xg[:, kk, :], start=st, stop=sp)
                            nc.tensor.matmul(acc_q[:], wq_sb[:, kc, 0:128],
                                             xg[:, kk, :], start=st, stop=sp)
                    rope_drain("k", n, acc_k)
                    rope_drain(0, n, acc_q)
                    v_drain(n, acc_v)

                ps1.release()

                # ================= pass 2: q1..q3 + attention + out-proj ====
                psp = tc.alloc_tile_pool(name="ps2", bufs=1, space="PSUM")

                def a_head(jj, h):
                    """Attention for chunk jj, head h. The k-block loop is
                    software-pipelined: pv/dn matmuls of block i are emitted
                    after stp/exp of block i+2, so the PE never waits on the
                    DVE/ACT softmax chain. Returns a deferred closure for the
                    final normalization (bc matmul + attn write); the caller
                    emits it after unrelated PE filler so the reciprocal
                    latency is hidden."""
                    jsl = slice(jj * 512, (jj + 1) * 512)
                    nblk = 4 * jj + 4 if mask_mode == "causal" else NKB
                    n_pairs = (4 * jj) // 2 if mask_mode == "causal" else NKB // 2
                    dn_total = n_pairs + 4 if mask_mode == "causal" else n_pairs
                    pv = psp.tile([128, 512], F32, tag="pv", bufs=1,
                                  name=f"pv{h}_{jj}")
                    dn = psp.tile([1, 512], F32, tag="dn", bufs=1,
                                  name=f"dn{h}_{jj}")
                    state = {"pending": None, "dn_i": 0}
                    fl = []

                    def flush_one():
                        i, pexp, off, diag = fl.pop(0)
                        nc.tensor.matmul(pv[:, off:],
                                         vs[:, i * 128:(i + 1) * 128],
                                         pexp[:, off:],
                                         start=(i == 0),
                                         stop=(i == nblk - 1))
                        dn_i = state["dn_i"]
                        if diag:
                            nc.tensor.matmul(dn[:, off:], ones_sb[:],
                                             pexp[:, off:],
                                             start=(dn_i == 0),
                                             stop=(dn_i == dn_total - 1))
                            state["dn_i"] += 1
                        elif state["pending"] is None:
                            state["pending"] = pexp
                        else:
                            pr = wp.tile([128, 512], BF, tag="ppair", bufs=3,
                                         name=f"pr{h}_{jj}_{i}")
                            nc.vector.tensor_add(pr[:], state["pending"][:],
                                                 pexp[:])
                            state["pending"] = None
                            nc.tensor.matmul(dn[:], ones_sb[:], pr[:],
                                             start=(dn_i == 0),
                                             stop=(dn_i == dn_total - 1))
                            state["dn_i"] += 1

                    for i in range(nblk):
                        r = i - 4 * jj
                        diag = mask_mode == "causal" and r >= 0
                        off = 128 * r if (diag and r > 0) else 0
                        stp = psp.tile([128, 512], F32, tag="big", bufs=3,
                                       name=f"st{h}_{jj}_{i}")
                        nc.tensor.matmul(stp[:, off:],
                                         kt[:, i * 128:(i + 1) * 128],
                                         qt[h][:, jj * 512 + off:(jj + 1) * 512],
                                         start=True, stop=True)
                        if mask_mode == "full":
                            mt = wp.tile([128, 512], F32, tag="mt", bufs=3)
                            nc.sync.dma_start(
                                mt[:], maskT_d[i * 128:(i + 1) * 128, jsl])
                            nc.vector.tensor_add(stp[:], stp[:], mt[:])
                        pexp = wp.tile([128, 512], BF, tag="pexp", bufs=6,
                                       name=f"pexp{h}_{jj}_{i}")
                        nc.scalar.activation(pexp[:, off:], stp[:, off:],
                                             EXP, scale=SCALE)
                        if diag:
                            nc.vector.tensor_mul(pexp[:, off:off + 128],
                                                 pexp[:, off:off + 128],
                                                 trimask_sb[:])
                        fl.append((i, pexp, off, diag))
                        if len(fl) > 2:
                            flush_one()
                    while fl:
                        flush_one()
                    assert state["pending"] is None and state["dn_i"] == dn_total
                    # reciprocal now (frees the dn bank); the broadcast matmul
                    # and attn write are deferred to hide the DVE latency
                    rcp = wp.tile([1, 512], F32, tag="rcp", bufs=2)
                    nc.vector.reciprocal_approx_fast(rcp[:], dn[:])
                    rcpr = wp.tile([1, 512], BF, tag="rcpr", bufs=2)
                    nc.vector.tensor_copy(rcpr[:], rcp[:])

                    def finalize():
                        bc = psp.tile([128, 512], F32, tag="dn", bufs=1,
                                      name=f"bc{h}_{jj}")
                        nc.tensor.matmul(bc[:], onesrow[:], rcpr[:],
                                         start=True, stop=True)
                        bcs = wp.tile([128, 512], BF, tag="bcs", bufs=2)
                        nc.scalar.copy(bcs[:], bc[:])
                        nc.vector.tensor_mul(attn[h][:, jsl], pv[:], bcs[:])
                    return finalize

                def wo_block(m, eng_flip):
                    # one 128-row seq block of the output projection
                    msl = slice(m * 128, (m + 1) * 128)
                    for w4 in range(4):
                        yps = [psp.tile([128, 512], F32, tag="big", bufs=3,
                                        name=f"yp{m}_{w4}_{i}")
                               for i in range(2)]
                        for kc in range(HPC):
                            for i in range(2):
                                ncol = w4 * 2 + i
                                nc.tensor.matmul(
                                    yps[i][:], attn[kc][:, msl],
                                    wo_sb[:, kc, ncol * 512:(ncol + 1) * 512],
                                    start=(kc == 0), stop=(kc == HPC - 1))
                        for i in range(2):
                            ncol = w4 * 2 + i
                            ysb = wp.tile([128, 512], BF, tag="ysb", bufs=6)
                            if (w4 + i + eng_flip) % 2 == 0:
                                nc.scalar.copy(ysb[:], yps[i][:])
                            else:
                                nc.vector.tensor_copy(ysb[:], yps[i][:])
                            nc.sync.dma_start(
                                out[msl, ncol * 512:(ncol + 1) * 512],
                                ysb[:])

                def p2_group(j, g, accs, nsl):
                    xg = wp.tile([128, XG, 512], BF, tag="xg", bufs=4,
                                 name=f"xg2_{j}_{g}")
                    nc.scalar.dma_start(
                        xg[:], xT3[g * XG:(g + 1) * XG, :, nsl]
                        .rearrange("kc p s -> p kc s"))
                    for kk in range(XG):
                        kc = g * XG + kk
                        st, sp = (kc == 0), (kc == NKC - 1)
                        for s, head in enumerate((1, 2, 3)):
                            nc.tensor.matmul(
                                accs[s],
                                wq_sb[:, kc, head * 128:(head + 1) * 128],
                                xg[:, kk, :], start=st, stop=sp)

                # ---- pass 2 main loop ----
                deferred = []  # pending attention-normalize closures

                def flush_deferred():
                    while deferred:
                        deferred.pop(0)()

                for j in range(NQC):
                    nsl = slice(j * 512, (j + 1) * 512)
                    jj, cc = j - 1, j - 2
                    accs = [psp.tile([128, 512], F32, tag="acc", bufs=3,
                                     name=f"acc2_{j}_{s}")[:]
                            for s in range(3)]
                    for h in range(HPC):
                        for g in range(4 * h, 4 * h + 4):
                            p2_group(j, g, accs, nsl)
                        if j == 0 and h == 0:
                            # wo load deferred to here: overlaps pass-2 compute
                            for gg in range(2):
                                nc.sync.dma_start(
                                    wo_sb[:, :, gg * 2048:(gg + 1) * 2048],
                                    wo3[:, :, gg * 2048:(gg + 1) * 2048])
                        flush_deferred()
                        if cc >= 0:
                            wo_block(4 * cc + h, h)
                        if jj >= 0:
                            deferred.append(a_head(jj, h))
                    for s, head in enumerate((1, 2, 3)):
                        rope_drain2(head, j, accs[s], psp)

                # ---- tail: attention chunk 3 + out-proj chunks 2,3 ----
                for h in range(HPC):
                    flush_deferred()
                    wo_block(8 + h, h)
                    deferred.append(a_head(NQC - 1, h))
                for m in range(12, 16):
                    flush_deferred()
                    wo_block(m, m)
                psp.release()

    nc.compile()
    return nc


def get_program(mask_mode: str):
    if mask_mode not in _PROG_CACHE:
        _PROG_CACHE[mask_mode] = _build_program(mask_mode)
    return _PROG_CACHE[mask_mode]


# ====================== host-side preparation ======================

_PERM128 = np.concatenate([np.arange(0, 128, 2), np.arange(1, 128, 2)])


def _bf16(a: np.ndarray) -> np.ndarray:
    import ml_dtypes
    return np.ascontiguousarray(a.astype(np.float32).astype(ml_dtypes.bfloat16))


def _perm_cols(w: np.ndarray, n_heads: int) -> np.ndarray:
    """Permute each head's 128 columns: even dims first, odd dims last."""
    cols = np.concatenate([h * 128 + _PERM128 for h in range(n_heads)])
    return w[:, cols]


def _classify_mask(mask: np.ndarray) -> str:
    if not np.any(mask):
        return "none"
    iu = np.triu_indices(SEQ, 1)
    upper = mask[iu]
    lower_ok = not np.any(np.tril(mask))
    upper_ok = bool(np.all(np.isneginf(upper) | (upper <= -1e9)))
    if lower_ok and upper_ok:
        return "causal"
    return "full"


def _host_inputs(x, wq, wk, wv, wo, freqs_cos, freqs_sin, mask):
    x2 = np.ascontiguousarray(x.reshape(SEQ, DIM).T)        # xT [DIM, SEQ]
    wq_p = _perm_cols(np.asarray(wq, np.float32), N_HEADS)
    wk_p = _perm_cols(np.asarray(wk, np.float32), N_KV)
    wv_ = np.asarray(wv, np.float32)
    wo_ = np.asarray(wo, np.float32)

    cosT = np.asarray(freqs_cos, np.float32).T              # [64, SEQ]
    sinT = np.asarray(freqs_sin, np.float32).T
    cos2 = np.concatenate([cosT, cosT], 0)                  # [128, SEQ]
    sin2 = np.concatenate([sinT, sinT], 0)

    rmat = np.zeros((HD, HD), np.float32)
    rmat[np.arange(64) + 64, np.arange(64)] = -1.0   # swp[:64] = -raw[64:]
    rmat[np.arange(64), np.arange(64) + 64] = 1.0    # swp[64:] = raw[:64]
    ident = np.eye(128, dtype=np.float32)

    mask = np.asarray(mask, np.float32)
    mode = _classify_mask(mask)

    common = {"xT": _bf16(x2), "cos2": _bf16(cos2), "sin2": _bf16(sin2),
              "rmat": _bf16(rmat), "ident": _bf16(ident),
              "ones_col": _bf16(np.ones((HD, 1), np.float32)),
              "ones_row": _bf16(np.ones((1, HD), np.float32))}
    if mode == "causal":
        kk = np.arange(128)[:, None]
        qq = np.arange(128)[None, :]
        common["trimask"] = _bf16((kk <= qq).astype(np.float32))
    elif mode == "full":
        m = np.where(np.isneginf(mask), NEG, mask)
        common["maskT"] = np.ascontiguousarray(m.T)

    in_maps = []
    for c in range(NCORES):
        im = dict(common)
        im["wq"] = _bf16(wq_p[:, c * QD:(c + 1) * QD])
        im["wk"] = _bf16(wk_p[:, c * HD:(c + 1) * HD])
        im["wv"] = _bf16(wv_[:, c * HD:(c + 1) * HD])
        im["wo"] = _bf16(wo_[c * QD:(c + 1) * QD, :])
        in_maps.append(im)
    return mode, in_maps


def _scores_safe(x, wq, wk):
    """The device softmax skips the max-subtraction (scores from
    setup_inputs()-scaled weights are O(5), so exp() is exact and safe).
    Estimate the score magnitude; if exp could overflow fp32, fall back."""
    sx = float(np.sqrt(np.mean(np.square(x), dtype=np.float64)))
    sq = sx * float(np.sqrt(np.mean(np.square(wq), dtype=np.float64)) * np.sqrt(DIM))
    sk = sx * float(np.sqrt(np.mean(np.square(wk), dtype=np.float64)) * np.sqrt(DIM))
    # rope with arbitrary freqs can scale q/k by ~sqrt(2); 7 sigma tail margin
    return 2.0 * sq * sk * 7.0 < 80.0


def _numpy_fallback(x, wq, wk, wv, wo, freqs_cos, freqs_sin, mask):
    """Slow but numerically-safe host path (stable softmax), used only when
    the score magnitudes could overflow the device's unshifted exp."""
    x2 = x.reshape(SEQ, DIM).astype(np.float64)
    q = (x2 @ wq.astype(np.float64)).reshape(SEQ, N_HEADS, HD)
    k = (x2 @ wk.astype(np.float64)).reshape(SEQ, N_KV, HD)
    v = (x2 @ wv.astype(np.float64)).reshape(SEQ, N_KV, HD)
    cos = freqs_cos.astype(np.float64)[:, None, :]
    sin = freqs_sin.astype(np.float64)[:, None, :]

    def rope(t):
        a, b = t[..., 0::2], t[..., 1::2]
        out = np.empty_like(t)
        out[..., 0::2] = a * cos - b * sin
        out[..., 1::2] = a * sin + b * cos
        return out

    q, k = rope(q), rope(k)
    m64 = mask.astype(np.float64)
    outh = np.empty((SEQ, N_HEADS, HD))
    for h in range(N_HEADS):
        g = h // (N_HEADS // N_KV)
        s = q[:, h, :] @ k[:, g, :].T / math.sqrt(HD) + m64
        p = np.exp(s - s.max(-1, keepdims=True))
        p /= p.sum(-1, keepdims=True)
        outh[:, h, :] = p @ v[:, g, :]
    y = outh.reshape(SEQ, N_HEADS * HD) @ wo.astype(np.float64)
    return y.astype(np.float32).reshape(1, SEQ, DIM)


def kernel(x, wq, wk, wv, wo, freqs_cos, freqs_sin, mask, cache_k, cache_v,
           start_pos, **_unused):
    sp = int(np.asarray(start_pos))
    x = np.asarray(x, np.float32)
    wq = np.asarray(wq, np.float32)
    wk = np.asarray(wk, np.float32)
    wv = np.asarray(wv, np.float32)
    wo = np.asarray(wo, np.float32)
    mask = np.asarray(mask, np.float32)
    if sp != 0:
        raise NotImplementedError("kernel assumes start_pos == 0 prefill")
    if not _scores_safe(x, wq, wk):
        return _numpy_fallback(x, wq, wk, wv, wo,
                               np.asarray(freqs_cos, np.float32),
                               np.asarray(freqs_sin, np.float32), mask)

    mode, in_maps = _host_inputs(x, wq, wk, wv, wo,
                                 freqs_cos, freqs_sin, mask)
    nc = get_program(mode)
    res = bass_utils.run_bass_kernel_spmd(nc, in_maps,
                                          core_ids=list(range(NCORES)))
    acc = np.zeros((SEQ, DIM), np.float64)
    for r in res.results:
        acc += r["out"].astype(np.float64)
    return acc.astype(np.float32).reshape(1, SEQ, DIM)


# revision 17
# speedup vs baseline: 1.1615x; 1.0771x over previous
"""Trainium2 Bass kernel: Llama-style attention block (prefill, start_pos=0).

Reference computation (per problem):
  q = x @ wq; k = x @ wk; v = x @ wv          (DIM=4096 -> 32 q-heads / 8 kv-heads, hd=128)
  rope(q, k) with interleaved (even, odd) pairs using freqs_cos/freqs_sin inputs
  scores = q @ k^T / sqrt(128) + mask ; p = softmax(scores) ; o = p @ v (GQA 4x)
  out = o @ wo

Distribution: tensor-parallel over heads on 8 cores. Core c owns q-heads
4c..4c+3 and kv-head c (GQA groups align with the core boundary), i.e.
wq/wk/wv are sharded column-wise and wo row-wise. Each core computes a
full-shape partial of the output projection; the host sums the 8 partials
(the row-parallel all-reduce, done on the host at unshard time).

Layout strategy on-chip (per core):
  - host passes x transposed (xT [4096, 2048]) so Q^T/K^T/V^T come out of the
    PE in dim-major layout [dims, seq], which is exactly the operand layout
    attention needs (contraction over head_dim = partition axis).
  - RoPE: wq/wk columns are permuted on the host so each head's rotation
    pairs (even, odd) become (first 64, last 64) rows. The pair swap
    [a;b] -> [-b;a] is then a constant 128x128 matmul on the PE, and the
    cos/sin combine is 3 elementwise DVE ops. Dot products are invariant
    under the permutation so scores match the reference exactly.
  - scores are computed transposed (S^T [k, q] blocks): exp runs on the
    scalar engine reading PSUM directly, and P^T feeds the P@V matmul with
    no transposes anywhere. V is transposed to seq-major once.
  - everything is bf16 (PSUM accumulation stays fp32): halves HBM traffic
    and doubles DVE elementwise throughput vs fp32; measured end-to-end
    error ~4e-3 vs the 2e-2 gate.
  - softmax denominators: pairs of pexp k-blocks are summed on the DVE
    (single bf16 rounding each) and contracted with a ones-vector matmul
    per pair; diagonal (causally-partial) blocks get their own ones-matmul.
    The causal mask inside a diagonal block is a [128,128] 0/1 bf16
    multiply on the DVE (upper blocks are skipped entirely).
  - pipeline: pass 1 computes K^T/V^T and q-head 0 for all chunks (3 PSUM
    accumulation banks); pass 2 per chunk j emits the q1..q3 projections
    interleaved with attention for chunk j-1 and the output projection for
    chunk j-2, so the PE never drains around the softmax latency chains.
"""

import math

import numpy as np

import concourse.bass as bass
import concourse.mybir as mybir
import concourse.tile as tile
from concourse import bacc, bass_utils

DIM = 4096
N_HEADS = 32
N_KV = 8
HD = 128
SEQ = 2048
NCORES = 8
HPC = N_HEADS // NCORES          # q heads per core
QD = HPC * HD                    # 512 q-dims per core
SCALE = 1.0 / math.sqrt(HD)
NEG = -1.0e30

NQC = SEQ // 512                 # q chunks of 512
NKB = SEQ // 128                 # k blocks of 128
NKC = DIM // 128                 # contraction chunks of 128
XG = 4                           # kc chunks per x DMA group
NGRP = NKC // XG                 # 8 x-groups per chunk

F32 = mybir.dt.float32
BF = mybir.dt.bfloat16
EXP = mybir.ActivationFunctionType.Exp

_PROG_CACHE = {}


def _build_program(mask_mode: str):
    """mask_mode: 'causal' (skip upper blocks, multiplicative triangular
    diagonal mask), 'none' (no masking), 'full' (add arbitrary maskT)."""
    assert mask_mode in ("causal", "none", "full")
    nc = bacc.Bacc("TRN2", target_bir_lowering=False, debug=False,
                   num_devices=NCORES)

    # all operand tensors arrive pre-packed in SBUF layout (partition-major)
    # so every load is a fully-contiguous DMA with multi-KB lines
    xp = nc.dram_tensor("xp", [128, NQC, NGRP, XG, 512], BF,
                        kind="ExternalInput").ap()
    wq01 = nc.dram_tensor("wq01", [128, NKC, 256], BF,
                          kind="ExternalInput").ap()
    wq23 = nc.dram_tensor("wq23", [128, NKC, 256], BF,
                          kind="ExternalInput").ap()
    wk = nc.dram_tensor("wk", [128, NKC, HD], BF, kind="ExternalInput").ap()
    wv = nc.dram_tensor("wv", [128, NKC, HD], BF, kind="ExternalInput").ap()
    wo = nc.dram_tensor("wo", [128, HPC, DIM], BF, kind="ExternalInput").ap()
    cos2 = nc.dram_tensor("cos2", [HD, SEQ], BF, kind="ExternalInput").ap()
    sin2 = nc.dram_tensor("sin2", [HD, SEQ], BF, kind="ExternalInput").ap()
    rmat = nc.dram_tensor("rmat", [HD, HD], BF, kind="ExternalInput").ap()
    ident = nc.dram_tensor("ident", [128, 128], BF, kind="ExternalInput").ap()
    ones_col_d = nc.dram_tensor("ones_col", [128, 1], BF,
                                kind="ExternalInput").ap()
    ones_row_d = nc.dram_tensor("ones_row", [1, 128], BF,
                                kind="ExternalInput").ap()
    if mask_mode == "causal":
        trimask_d = nc.dram_tensor("trimask", [128, 128], BF,
                                   kind="ExternalInput").ap()
    if mask_mode == "full":
        maskT_d = nc.dram_tensor("maskT", [SEQ, SEQ], F32,
                                 kind="ExternalInput").ap()
    out = nc.dram_tensor("out", [SEQ, DIM], BF, kind="ExternalOutput").ap()

    with tile.TileContext(nc) as tc:
        with tc.tile_pool(name="persist", bufs=1) as pp:
            # ---- persistent tiles ----
            qt = [pp.tile([128, SEQ], BF, name=f"qt{h}") for h in range(HPC)]
            kt = pp.tile([128, SEQ], BF)
            vs = pp.tile([128, SEQ], BF)         # seq-major V, block i at cols i*128
            attn = [pp.tile([128, SEQ], BF, name=f"attn{h}")
                    for h in range(HPC)]
            rmat_sb = pp.tile([128, 128], BF)
            ident_sb = pp.tile([128, 128], BF)
            ones_sb = pp.tile([128, 1], BF)
            onesrow = pp.tile([1, 128], BF)
            nc.gpsimd.dma_start(ident_sb[:], ident[:])
            nc.gpsimd.dma_start(rmat_sb[:], rmat[:])
            nc.gpsimd.dma_start(ones_sb[:], ones_col_d[:])
            nc.gpsimd.dma_start(onesrow[:], ones_row_d[:])
            if mask_mode == "causal":
                trimask_sb = pp.tile([128, 128], BF)
                nc.gpsimd.dma_start(trimask_sb[:], trimask_d[:])
            cos_sb = pp.tile([128, SEQ], BF)
            sin_sb = pp.tile([128, SEQ], BF)
            wq01_sb = pp.tile([128, NKC, 256], BF)
            wq23_sb = pp.tile([128, NKC, 256], BF)
            wk_sb = pp.tile([128, NKC, HD], BF)
            wv_sb = pp.tile([128, NKC, HD], BF)
            wo_sb = pp.tile([128, HPC, DIM], BF)

            # pass-1 weight loads (wk, wv, q-heads 0/1) then cos/sin; the
            # second wq half streams during pass 1 (needed at pass-2 start)
            # and wo during early pass 2 (first needed ~160us in), keeping
            # startup HBM bandwidth for x.
            nc.sync.dma_start(wk_sb[:, 0:8, :], wk[:, 0:8, :])
            nc.sync.dma_start(wv_sb[:, 0:8, :], wv[:, 0:8, :])
            nc.sync.dma_start(wq01_sb[:, 0:8, :], wq01[:, 0:8, :])
            nc.sync.dma_start(wk_sb[:, 8:32, :], wk[:, 8:32, :])
            nc.sync.dma_start(wv_sb[:, 8:32, :], wv[:, 8:32, :])
            nc.sync.dma_start(wq01_sb[:, 8:32, :], wq01[:, 8:32, :])
            nc.sync.dma_start(cos_sb[:], cos2[:])
            nc.sync.dma_start(sin_sb[:], sin2[:])
            nc.sync.dma_start(wq23_sb[:], wq23[:])

            # ================= pass 1: K, V, q-heads 0/1 =================
            ps1 = tc.alloc_tile_pool(name="ps1", bufs=1, space="PSUM")
            # dummy matmuls on the identity tile keep the PE activity monitor
            # warm through the initial DMA window (else the first ~3.5us of
            # real matmuls run at half clock)
            warm = ps1.tile([128, 128], F32, tag="aux", bufs=2)
            for _ in range(36):
                nc.tensor.matmul(warm[:], ident_sb[:], ident_sb[:],
                                 start=True, stop=True)

            with tc.tile_pool(name="work", bufs=1) as wp:

                def rope_drain(head_or_k, n, acc):
                    # acc [128,512] f32 PSUM -> rope -> qt[h]/kt bf16
                    nsl = slice(n * 512, (n + 1) * 512)
                    dst = kt if head_or_k == "k" else qt[head_or_k]
                    raw = wp.tile([128, 512], BF, tag="raw", bufs=4,
                                  name=f"raw{head_or_k}_{n}")
                    nc.scalar.copy(raw[:], acc[:])
                    swp = ps1.tile([128, 512], F32, tag="aux", bufs=2,
                                   name=f"swp{head_or_k}_{n}")
                    nc.tensor.matmul(swp[:], rmat_sb[:], raw[:],
                                     start=True, stop=True)
                    nc.vector.tensor_mul(dst[:, nsl], swp[:], sin_sb[:, nsl])
                    tmp = wp.tile([128, 512], BF, tag="ropetmp", bufs=2,
                                  name=f"tmp{head_or_k}_{n}")
                    nc.vector.tensor_mul(tmp[:], raw[:], cos_sb[:, nsl])
                    nc.vector.tensor_add(dst[:, nsl], dst[:, nsl], tmp[:])

                def rope_drain2(head_or_k, n, acc, psp):
                    # same but allocating the swap tile from the pass-2 pool
                    nsl = slice(n * 512, (n + 1) * 512)
                    dst = kt if head_or_k == "k" else qt[head_or_k]
                    raw = wp.tile([128, 512], BF, tag="raw", bufs=4,
                                  name=f"raw{head_or_k}_{n}")
                    nc.scalar.copy(raw[:], acc[:])
                    swp = psp.tile([128, 512], F32, tag="big", bufs=3,
                                   name=f"swp{head_or_k}_{n}")
                    nc.tensor.matmul(swp[:], rmat_sb[:], raw[:],
                                     start=True, stop=True)
                    nc.vector.tensor_mul(dst[:, nsl], swp[:], sin_sb[:, nsl])
                    tmp = wp.tile([128, 512], BF, tag="ropetmp", bufs=2,
                                  name=f"tmp{head_or_k}_{n}")
                    nc.vector.tensor_mul(tmp[:], raw[:], cos_sb[:, nsl])
                    nc.vector.tensor_add(dst[:, nsl], dst[:, nsl], tmp[:])

                def v_drain(n, acc):
                    raw = wp.tile([128, 512], BF, tag="raw", bufs=4,
                                  name=f"rawv_{n}")
                    nc.vector.tensor_copy(raw[:], acc[:])
                    vtr = ps1.tile([128, 512], BF, tag="aux", bufs=2,
                                   name=f"vtr_{n}")
                    for b in range(4):
                        nc.tensor.transpose(vtr[:, b * 128:(b + 1) * 128],
                                            raw[:, b * 128:(b + 1) * 128],
                                            ident_sb[:])
                    nc.scalar.copy(vs[:, n * 512:(n + 1) * 512], vtr[:])

                # ---- pass 1 main loop ----
                for n in range(NQC):
                    acc_k = ps1.tile([128, 512], F32, tag="acc", bufs=5,
                                     name=f"acck_{n}")
                    acc_v = ps1.tile([128, 512], F32, tag="acc", bufs=5,
                                     name=f"accv_{n}")
                    acc_q0 = ps1.tile([128, 512], F32, tag="acc", bufs=5,
                                      name=f"accq0_{n}")
                    acc_q1 = ps1.tile([128, 512], F32, tag="acc", bufs=5,
                                      name=f"accq1_{n}")
                    for g in range(NGRP):
                        xg = wp.tile([128, XG, 512], BF, tag="xg", bufs=4,
                                     name=f"xg1_{n}_{g}")
                        nc.scalar.dma_start(xg[:], xp[:, n, g, :, :])
                        for kk in range(XG):
                            kc = g * XG + kk
                            st, sp = (kc == 0), (kc == NKC - 1)
                            nc.tensor.matmul(acc_k[:], wk_sb[:, kc, :],
                                             xg[:, kk, :], start=st, stop=sp)
                            nc.tensor.matmul(acc_v[:], wv_sb[:, kc, :],
                                             xg[:, kk, :], start=st, stop=sp)
                            nc.tensor.matmul(acc_q0[:], wq01_sb[:, kc, 0:128],
                                             xg[:, kk, :], start=st, stop=sp)
                            nc.tensor.matmul(acc_q1[:], wq01_sb[:, kc, 128:256],
                                             xg[:, kk, :], start=st, stop=sp)
                    rope_drain("k", n, acc_k)
                    rope_drain(0, n, acc_q0)
                    rope_drain(1, n, acc_q1)
                    v_drain(n, acc_v)

                ps1.release()

                # ================= pass 2: q1..q3 + attention + out-proj ====
                psp = tc.alloc_tile_pool(name="ps2", bufs=1, space="PSUM")

                def a_head(jj, h):
                    """Attention for chunk jj, head h. The k-block loop is
                    software-pipelined: pv/dn matmuls of block i are emitted
                    after stp/exp of block i+2, so the PE never waits on the
                    DVE/ACT softmax chain. Returns a deferred closure for the
                    final normalization (bc matmul + attn write); the caller
                    emits it after unrelated PE filler so the reciprocal
                    latency is hidden."""
                    jsl = slice(jj * 512, (jj + 1) * 512)
                    nblk = 4 * jj + 4 if mask_mode == "causal" else NKB
                    n_pairs = (4 * jj) // 2 if mask_mode == "causal" else NKB // 2
                    dn_total = n_pairs + 4 if mask_mode == "causal" else n_pairs
                    pv = psp.tile([128, 512], F32, tag="pv", bufs=1,
                                  name=f"pv{h}_{jj}")
                    dn = psp.tile([1, 512], F32, tag="dn", bufs=1,
                                  name=f"dn{h}_{jj}")
                    state = {"pending": None, "dn_i": 0}
                    fl = []

                    def flush_one():
                        i, pexp, off, diag = fl.pop(0)
                        nc.tensor.matmul(pv[:, off:],
                                         vs[:, i * 128:(i + 1) * 128],
                                         pexp[:, off:],
                                         start=(i == 0),
                                         stop=(i == nblk - 1))
                        dn_i = state["dn_i"]
                        if diag:
                            nc.tensor.matmul(dn[:, off:], ones_sb[:],
                                             pexp[:, off:],
                                             start=(dn_i == 0),
                                             stop=(dn_i == dn_total - 1))
                            state["dn_i"] += 1
                        elif state["pending"] is None:
                            state["pending"] = pexp
                        else:
                            pr = wp.tile([128, 512], BF, tag="ppair", bufs=3,
                                         name=f"pr{h}_{jj}_{i}")
                            nc.vector.tensor_add(pr[:], state["pending"][:],
                                                 pexp[:])
                            state["pending"] = None
                            nc.tensor.matmul(dn[:], ones_sb[:], pr[:],
                                             start=(dn_i == 0),
                                             stop=(dn_i == dn_total - 1))
                            state["dn_i"] += 1

                    for i in range(nblk):
                        r = i - 4 * jj
                        diag = mask_mode == "causal" and r >= 0
                        off = 128 * r if (diag and r > 0) else 0
                        stp = psp.tile([128, 512], F32, tag="big", bufs=3,
                                       name=f"st{h}_{jj}_{i}")
                        nc.tensor.matmul(stp[:, off:],
                                         kt[:, i * 128:(i + 1) * 128],
                                         qt[h][:, jj * 512 + off:(jj + 1) * 512],
                                         start=True, stop=True)
                        if mask_mode == "full":
                            mt = wp.tile([128, 512], F32, tag="mt", bufs=3)
                            nc.sync.dma_start(
                                mt[:], maskT_d[i * 128:(i + 1) * 128, jsl])
                            nc.vector.tensor_add(stp[:], stp[:], mt[:])
                        pexp = wp.tile([128, 512], BF, tag="pexp", bufs=6,
                                       name=f"pexp{h}_{jj}_{i}")
                        nc.scalar.activation(pexp[:, off:], stp[:, off:],
                                             EXP, scale=SCALE)
                        if diag:
                            nc.vector.tensor_mul(pexp[:, off:off + 128],
                                                 pexp[:, off:off + 128],
                                                 trimask_sb[:])
                        fl.append((i, pexp, off, diag))
                        if len(fl) > 2:
                            flush_one()
                    while fl:
                        flush_one()
                    assert state["pending"] is None and state["dn_i"] == dn_total
                    # reciprocal now (frees the dn bank); the broadcast matmul
                    # and attn write are deferred to hide the DVE latency
                    rcp = wp.tile([1, 512], F32, tag="rcp", bufs=2)
                    nc.vector.reciprocal_approx_fast(rcp[:], dn[:])
                    rcpr = wp.tile([1, 512], BF, tag="rcpr", bufs=2)
                    nc.vector.tensor_copy(rcpr[:], rcp[:])

                    def finalize():
                        bc = psp.tile([128, 512], F32, tag="dn", bufs=1,
                                      name=f"bc{h}_{jj}")
                        nc.tensor.matmul(bc[:], onesrow[:], rcpr[:],
                                         start=True, stop=True)
                        bcs = wp.tile([128, 512], BF, tag="bcs", bufs=2)
                        nc.scalar.copy(bcs[:], bc[:])
                        nc.vector.tensor_mul(attn[h][:, jsl], pv[:], bcs[:])
                    return finalize

                def wo_block(m, eng_flip):
                    # one 128-row seq block of the output projection
                    msl = slice(m * 128, (m + 1) * 128)
                    for w4 in range(4):
                        yps = [psp.tile([128, 512], F32, tag="big", bufs=3,
                                        name=f"yp{m}_{w4}_{i}")
                               for i in range(2)]
                        for kc in range(HPC):
                            for i in range(2):
                                ncol = w4 * 2 + i
                                nc.tensor.matmul(
                                    yps[i][:], attn[kc][:, msl],
                                    wo_sb[:, kc, ncol * 512:(ncol + 1) * 512],
                                    start=(kc == 0), stop=(kc == HPC - 1))
                        for i in range(2):
                            ncol = w4 * 2 + i
                            ysb = wp.tile([128, 512], BF, tag="ysb", bufs=6)
                            if (w4 + i + eng_flip) % 2 == 0:
                                nc.scalar.copy(ysb[:], yps[i][:])
                            else:
                                nc.vector.tensor_copy(ysb[:], yps[i][:])
                            nc.sync.dma_start(
                                out[msl, ncol * 512:(ncol + 1) * 512],
                                ysb[:])

                def p2_group(j, g, accs):
                    xg = wp.tile([128, XG, 512], BF, tag="xg", bufs=4,
                                 name=f"xg2_{j}_{g}")
                    nc.scalar.dma_start(xg[:], xp[:, j, g, :, :])
                    for kk in range(XG):
                        kc = g * XG + kk
                        st, sp = (kc == 0), (kc == NKC - 1)
                        for s in range(2):
                            nc.tensor.matmul(
                                accs[s][:],
                                wq23_sb[:, kc, s * 128:(s + 1) * 128],
                                xg[:, kk, :], start=st, stop=sp)

                # ---- pass 2 main loop ----
                deferred = []  # pending attention-normalize closures

                def flush_deferred():
                    while deferred:
                        deferred.pop(0)()

                for j in range(NQC):
                    jj, cc = j - 1, j - 2
                    accs = [psp.tile([128, 512], F32, tag="acc", bufs=3,
                                     name=f"acc2_{j}_{s}")
                            for s in range(2)]
                    for h in range(HPC):
                        for g in range(2 * h, 2 * h + 2):
                            p2_group(j, g, accs)
                        if j == 0 and h == 0:
                            # wo load deferred to here: overlaps pass-2 compute
                            for gg in range(2):
                                nc.sync.dma_start(
                                    wo_sb[:, :, gg * 2048:(gg + 1) * 2048],
                                    wo[:, :, gg * 2048:(gg + 1) * 2048])
                        flush_deferred()
                        if cc >= 0:
                            wo_block(4 * cc + h, h)
                        if jj >= 0:
                            deferred.append(a_head(jj, h))
                    for s, head in enumerate((2, 3)):
                        rope_drain2(head, j, accs[s], psp)

                # ---- tail: attention chunk 3 + out-proj chunks 2,3 ----
                for h in range(HPC):
                    flush_deferred()
                    wo_block(8 + h, h)
                    deferred.append(a_head(NQC - 1, h))
                for m in range(12, 16):
                    flush_deferred()
                    wo_block(m, m)
                psp.release()

    nc.compile()
    return nc


def get_program(mask_mode: str):
    if mask_mode not in _PROG_CACHE:
        _PROG_CACHE[mask_mode] = _build_program(mask_mode)
    return _PROG_CACHE[mask_mode]


# ====================== host-side preparation ======================

_PERM128 = np.concatenate([np.arange(0, 128, 2), np.arange(1, 128, 2)])


def _bf16(a: np.ndarray) -> np.ndarray:
    import ml_dtypes
    return np.ascontiguousarray(a.astype(np.float32).astype(ml_dtypes.bfloat16))


def _perm_cols(w: np.ndarray, n_heads: int) -> np.ndarray:
    """Permute each head's 128 columns: even dims first, odd dims last."""
    cols = np.concatenate([h * 128 + _PERM128 for h in range(n_heads)])
    return w[:, cols]


def _classify_mask(mask: np.ndarray) -> str:
    if not np.any(mask):
        return "none"
    iu = np.triu_indices(SEQ, 1)
    upper = mask[iu]
    lower_ok = not np.any(np.tril(mask))
    upper_ok = bool(np.all(np.isneginf(upper) | (upper <= -1e9)))
    if lower_ok and upper_ok:
        return "causal"
    return "full"


def _pack_w(w: np.ndarray) -> np.ndarray:
    """[DIM, M] -> SBUF layout [128, NKC, M] (partition-major, contiguous)."""
    m = w.shape[1]
    return _bf16(w.reshape(NKC, 128, m).transpose(1, 0, 2))


def _host_inputs(x, wq, wk, wv, wo, freqs_cos, freqs_sin, mask):
    wq_p = _perm_cols(np.asarray(wq, np.float32), N_HEADS)
    wk_p = _perm_cols(np.asarray(wk, np.float32), N_KV)
    wv_ = np.asarray(wv, np.float32)
    wo_ = np.asarray(wo, np.float32)

    # x packed to [128, NQC, NGRP, XG, 512]: xp[p,n,g,kk,s] = x[n*512+s,
    # (g*XG+kk)*128+p] — every DMA group is one contiguous multi-KB run.
    x2 = np.asarray(x, np.float32).reshape(SEQ, DIM)
    xpk = _bf16(x2.reshape(NQC, 512, NGRP, XG, 128)
                .transpose(4, 0, 2, 3, 1))

    cosT = np.asarray(freqs_cos, np.float32).T              # [64, SEQ]
    sinT = np.asarray(freqs_sin, np.float32).T
    cos2 = np.concatenate([cosT, cosT], 0)                  # [128, SEQ]
    sin2 = np.concatenate([sinT, sinT], 0)

    rmat = np.zeros((HD, HD), np.float32)
    rmat[np.arange(64) + 64, np.arange(64)] = -1.0   # swp[:64] = -raw[64:]
    rmat[np.arange(64), np.arange(64) + 64] = 1.0    # swp[64:] = raw[:64]
    ident = np.eye(128, dtype=np.float32)

    mask = np.asarray(mask, np.float32)
    mode = _classify_mask(mask)

    common = {"xp": xpk, "cos2": _bf16(cos2), "sin2": _bf16(sin2),
              "rmat": _bf16(rmat), "ident": _bf16(ident),
              "ones_col": _bf16(np.ones((HD, 1), np.float32)),
              "ones_row": _bf16(np.ones((1, HD), np.float32))}
    if mode == "causal":
        kk = np.arange(128)[:, None]
        qq = np.arange(128)[None, :]
        common["trimask"] = _bf16((kk <= qq).astype(np.float32))
    elif mode == "full":
        m = np.where(np.isneginf(mask), NEG, mask)
        common["maskT"] = np.ascontiguousarray(m.T)

    in_maps = []
    for c in range(NCORES):
        im = dict(common)
        wq_c = wq_p[:, c * QD:(c + 1) * QD]
        im["wq01"] = _pack_w(wq_c[:, 0:256])
        im["wq23"] = _pack_w(wq_c[:, 256:512])
        im["wk"] = _pack_w(wk_p[:, c * HD:(c + 1) * HD])
        im["wv"] = _pack_w(wv_[:, c * HD:(c + 1) * HD])
        # wo packed to [128, HPC, DIM]: wo[p,kc,nn] = wo_[kc*128+p, nn]
        im["wo"] = _bf16(wo_[c * QD:(c + 1) * QD, :]
                         .reshape(HPC, 128, DIM).transpose(1, 0, 2))
        in_maps.append(im)
    return mode, in_maps


def _scores_safe(x, wq, wk):
    """The device softmax skips the max-subtraction (scores from
    setup_inputs()-scaled weights are O(5), so exp() is exact and safe).
    Estimate the score magnitude; if exp could overflow fp32, fall back."""
    sx = float(np.sqrt(np.mean(np.square(x), dtype=np.float64)))
    sq = sx * float(np.sqrt(np.mean(np.square(wq), dtype=np.float64)) * np.sqrt(DIM))
    sk = sx * float(np.sqrt(np.mean(np.square(wk), dtype=np.float64)) * np.sqrt(DIM))
    # rope with arbitrary freqs can scale q/k by ~sqrt(2); 7 sigma tail margin
    return 2.0 * sq * sk * 7.0 < 80.0


def _numpy_fallback(x, wq, wk, wv, wo, freqs_cos, freqs_sin, mask):
    """Slow but numerically-safe host path (stable softmax), used only when
    the score magnitudes could overflow the device's unshifted exp."""
    x2 = x.reshape(SEQ, DIM).astype(np.float64)
    q = (x2 @ wq.astype(np.float64)).reshape(SEQ, N_HEADS, HD)
    k = (x2 @ wk.astype(np.float64)).reshape(SEQ, N_KV, HD)
    v = (x2 @ wv.astype(np.float64)).reshape(SEQ, N_KV, HD)
    cos = freqs_cos.astype(np.float64)[:, None, :]
    sin = freqs_sin.astype(np.float64)[:, None, :]

    def rope(t):
        a, b = t[..., 0::2], t[..., 1::2]
        out = np.empty_like(t)
        out[..., 0::2] = a * cos - b * sin
        out[..., 1::2] = a * sin + b * cos
        return out

    q, k = rope(q), rope(k)
    m64 = mask.astype(np.float64)
    outh = np.empty((SEQ, N_HEADS, HD))
    for h in range(N_HEADS):
        g = h // (N_HEADS // N_KV)
        s = q[:, h, :] @ k[:, g, :].T / math.sqrt(HD) + m64
        p = np.exp(s - s.max(-1, keepdims=True))
        p /= p.sum(-1, keepdims=True)
        outh[:, h, :] = p @ v[:, g, :]
    y = outh.reshape(SEQ, N_HEADS * HD) @ wo.astype(np.float64)
    return y.astype(np.float32).reshape(1, SEQ, DIM)


def kernel(x, wq, wk, wv, wo, freqs_cos, freqs_sin, mask, cache_k, cache_v,
           start_pos, **_unused):
    sp = int(np.asarray(start_pos))
    x = np.asarray(x, np.float32)
    wq = np.asarray(wq, np.float32)
    wk = np.asarray(wk, np.float32)
    wv = np.asarray(wv, np.float32)
    wo = np.asarray(wo, np.float32)
    mask = np.asarray(mask, np.float32)
    if sp != 0:
        raise NotImplementedError("kernel assumes start_pos == 0 prefill")
    if not _scores_safe(x, wq, wk):
        return _numpy_fallback(x, wq, wk, wv, wo,
                               np.asarray(freqs_cos, np.float32),
                               np.asarray(freqs_sin, np.float32), mask)

    mode, in_maps = _host_inputs(x, wq, wk, wv, wo,
                                 freqs_cos, freqs_sin, mask)
    nc = get_program(mode)
    res = bass_utils.run_bass_kernel_spmd(nc, in_maps,
                                          core_ids=list(range(NCORES)))
    acc = np.zeros((SEQ, DIM), np.float64)
    for r in res.results:
        acc += r["out"].astype(np.float64)
    return acc.astype(np.float32).reshape(1, SEQ, DIM)


# revision 25
# speedup vs baseline: 1.1874x; 1.0223x over previous
"""Trainium2 Bass kernel: Llama-style attention block (prefill, start_pos=0).

Reference computation (per problem):
  q = x @ wq; k = x @ wk; v = x @ wv          (DIM=4096 -> 32 q-heads / 8 kv-heads, hd=128)
  rope(q, k) with interleaved (even, odd) pairs using freqs_cos/freqs_sin inputs
  scores = q @ k^T / sqrt(128) + mask ; p = softmax(scores) ; o = p @ v (GQA 4x)
  out = o @ wo

Distribution: tensor-parallel over heads on 8 cores. Core c owns q-heads
4c..4c+3 and kv-head c (GQA groups align with the core boundary), i.e.
wq/wk/wv are sharded column-wise and wo row-wise. Each core computes a
full-shape partial of the output projection; the host sums the 8 partials
(the row-parallel all-reduce, done on the host at unshard time).

Layout strategy on-chip (per core):
  - host passes x transposed (xT [4096, 2048]) so Q^T/K^T/V^T come out of the
    PE in dim-major layout [dims, seq], which is exactly the operand layout
    attention needs (contraction over head_dim = partition axis).
  - RoPE: wq/wk columns are permuted on the host so each head's rotation
    pairs (even, odd) become (first 64, last 64) rows. The pair swap
    [a;b] -> [-b;a] is then a constant 128x128 matmul on the PE, and the
    cos/sin combine is 3 elementwise DVE ops. Dot products are invariant
    under the permutation so scores match the reference exactly.
  - scores are computed transposed (S^T [k, q] blocks): exp runs on the
    scalar engine reading PSUM directly, and P^T feeds the P@V matmul with
    no transposes anywhere. V is transposed to seq-major once.
  - everything is bf16 (PSUM accumulation stays fp32): halves HBM traffic
    and doubles DVE elementwise throughput vs fp32; measured end-to-end
    error ~4e-3 vs the 2e-2 gate.
  - softmax denominators: pairs of pexp k-blocks are summed on the DVE
    (single bf16 rounding each) and contracted with a ones-vector matmul
    per pair; diagonal (causally-partial) blocks get their own ones-matmul.
    The causal mask inside a diagonal block is a [128,128] 0/1 bf16
    multiply on the DVE (upper blocks are skipped entirely).
  - pipeline: pass 1 computes K^T/V^T and q-head 0 for all chunks (3 PSUM
    accumulation banks); pass 2 per chunk j emits the q1..q3 projections
    interleaved with attention for chunk j-1 and the output projection for
    chunk j-2, so the PE never drains around the softmax latency chains.
"""

import math

import numpy as np

import concourse.bass as bass
import concourse.mybir as mybir
import concourse.tile as tile
from concourse import bacc, bass_utils

DIM = 4096
N_HEADS = 32
N_KV = 8
HD = 128
SEQ = 2048
NCORES = 8
HPC = N_HEADS // NCORES          # q heads per core
QD = HPC * HD                    # 512 q-dims per core
SCALE = 1.0 / math.sqrt(HD)
NEG = -1.0e30

NQC = SEQ // 512                 # q chunks of 512
NKB = SEQ // 128                 # k blocks of 128
NKC = DIM // 128                 # contraction chunks of 128
XG = 4                           # kc chunks per x DMA group
NGRP = NKC // XG                 # 8 x-groups per chunk

F32 = mybir.dt.float32
BF = mybir.dt.bfloat16
EXP = mybir.ActivationFunctionType.Exp

_PROG_CACHE = {}


def _build_program(mask_mode: str):
    """mask_mode: 'causal' (skip upper blocks, multiplicative triangular
    diagonal mask), 'none' (no masking), 'full' (add arbitrary maskT)."""
    assert mask_mode in ("causal", "none", "full")
    nc = bacc.Bacc("TRN2", target_bir_lowering=False, debug=False,
                   num_devices=NCORES)

    # all operand tensors arrive pre-packed in SBUF layout (partition-major)
    # so every load is a fully-contiguous DMA with multi-KB lines
    xp = nc.dram_tensor("xp", [128, NQC, NGRP, XG, 512], BF,
                        kind="ExternalInput").ap()
    wq01 = nc.dram_tensor("wq01", [128, NKC, 256], BF,
                          kind="ExternalInput").ap()
    wq23 = nc.dram_tensor("wq23", [128, NKC, 256], BF,
                          kind="ExternalInput").ap()
    wk = nc.dram_tensor("wk", [128, NKC, HD], BF, kind="ExternalInput").ap()
    wv = nc.dram_tensor("wv", [128, NKC, HD], BF, kind="ExternalInput").ap()
    wo = nc.dram_tensor("wo", [128, HPC, DIM], BF, kind="ExternalInput").ap()
    cos2 = nc.dram_tensor("cos2", [HD, SEQ], BF, kind="ExternalInput").ap()
    sin2 = nc.dram_tensor("sin2", [HD, SEQ], BF, kind="ExternalInput").ap()
    rmat = nc.dram_tensor("rmat", [HD, HD], BF, kind="ExternalInput").ap()
    ident = nc.dram_tensor("ident", [128, 128], BF, kind="ExternalInput").ap()
    ones_col_d = nc.dram_tensor("ones_col", [128, 1], BF,
                                kind="ExternalInput").ap()
    ones_row_d = nc.dram_tensor("ones_row", [1, 128], BF,
                                kind="ExternalInput").ap()
    if mask_mode == "causal":
        trimask_d = nc.dram_tensor("trimask", [128, 128], BF,
                                   kind="ExternalInput").ap()
    if mask_mode == "full":
        maskT_d = nc.dram_tensor("maskT", [SEQ, SEQ], F32,
                                 kind="ExternalInput").ap()
    out = nc.dram_tensor("out", [SEQ, DIM], BF, kind="ExternalOutput").ap()

    with tile.TileContext(nc) as tc:
        with tc.tile_pool(name="persist", bufs=1) as pp:
            # ---- persistent tiles ----
            qt = [pp.tile([128, SEQ], BF, name=f"qt{h}") for h in range(HPC)]
            kt = pp.tile([128, SEQ], BF)
            vs = pp.tile([128, SEQ], BF)         # seq-major V, block i at cols i*128
            attn = [pp.tile([128, SEQ], BF, name=f"attn{h}")
                    for h in range(HPC)]
            rmat_sb = pp.tile([128, 128], BF)
            ident_sb = pp.tile([128, 128], BF)
            ones_sb = pp.tile([128, 1], BF)
            onesrow = pp.tile([1, 128], BF)
            nc.gpsimd.dma_start(ident_sb[:], ident[:])
            nc.gpsimd.dma_start(rmat_sb[:], rmat[:])
            nc.gpsimd.dma_start(ones_sb[:], ones_col_d[:])
            nc.gpsimd.dma_start(onesrow[:], ones_row_d[:])
            if mask_mode == "causal":
                trimask_sb = pp.tile([128, 128], BF)
                nc.gpsimd.dma_start(trimask_sb[:], trimask_d[:])
            cos_sb = pp.tile([128, SEQ], BF)
            sin_sb = pp.tile([128, SEQ], BF)
            wq01_sb = pp.tile([128, NKC, 256], BF)
            wq23_sb = pp.tile([128, NKC, 256], BF)
            wk_sb = pp.tile([128, NKC, HD], BF)
            wv_sb = pp.tile([128, NKC, HD], BF)
            wo_sb = pp.tile([128, HPC, DIM], BF)

            # pass-1 weight loads (wk, wv, q-heads 0/1) then cos/sin; the
            # second wq half streams during pass 1 (needed at pass-2 start)
            # and wo during early pass 2 (first needed ~160us in), keeping
            # startup HBM bandwidth for x.
            nc.sync.dma_start(wk_sb[:, 0:8, :], wk[:, 0:8, :])
            nc.sync.dma_start(wv_sb[:, 0:8, :], wv[:, 0:8, :])
            nc.sync.dma_start(wq01_sb[:, 0:8, :], wq01[:, 0:8, :])
            nc.sync.dma_start(wk_sb[:, 8:32, :], wk[:, 8:32, :])
            nc.sync.dma_start(wv_sb[:, 8:32, :], wv[:, 8:32, :])
            nc.sync.dma_start(wq01_sb[:, 8:32, :], wq01[:, 8:32, :])
            nc.sync.dma_start(cos_sb[:], cos2[:])
            nc.sync.dma_start(sin_sb[:], sin2[:])
            nc.sync.dma_start(wq23_sb[:], wq23[:])

            # ================= pass 1: K, V, q-heads 0/1 =================
            ps1 = tc.alloc_tile_pool(name="ps1", bufs=1, space="PSUM")
            # dummy matmuls on the identity tile keep the PE activity monitor
            # warm through the initial DMA window (else the first ~3.5us of
            # real matmuls run at half clock)
            warm = ps1.tile([128, 128], F32, tag="aux", bufs=2)
            for _ in range(36):
                nc.tensor.matmul(warm[:], ident_sb[:], ident_sb[:],
                                 start=True, stop=True)

            with tc.tile_pool(name="work", bufs=1) as wp:

                def rope_drain(head_or_k, n, acc, dve_copy=False):
                    # acc [128,512] f32 PSUM -> rope -> qt[h]/kt bf16
                    nsl = slice(n * 512, (n + 1) * 512)
                    dst = kt if head_or_k == "k" else qt[head_or_k]
                    raw = wp.tile([128, 512], BF, tag="raw", bufs=4,
                                  name=f"raw{head_or_k}_{n}")
                    if dve_copy:
                        nc.vector.tensor_copy(raw[:], acc[:])
                    else:
                        nc.scalar.copy(raw[:], acc[:])
                    swp = ps1.tile([128, 512], F32, tag="aux", bufs=2,
                                   name=f"swp{head_or_k}_{n}")
                    nc.tensor.matmul(swp[:], rmat_sb[:], raw[:],
                                     start=True, stop=True)
                    nc.vector.tensor_mul(dst[:, nsl], swp[:], sin_sb[:, nsl])
                    tmp = wp.tile([128, 512], BF, tag="ropetmp", bufs=2,
                                  name=f"tmp{head_or_k}_{n}")
                    nc.vector.tensor_mul(tmp[:], raw[:], cos_sb[:, nsl])
                    nc.vector.tensor_add(dst[:, nsl], dst[:, nsl], tmp[:])

                def rope_drain2(head_or_k, n, acc, psp, dve_copy=False):
                    # same but allocating the swap tile from the pass-2 pool
                    nsl = slice(n * 512, (n + 1) * 512)
                    dst = kt if head_or_k == "k" else qt[head_or_k]
                    raw = wp.tile([128, 512], BF, tag="raw", bufs=4,
                                  name=f"raw{head_or_k}_{n}")
                    if dve_copy:
                        nc.vector.tensor_copy(raw[:], acc[:])
                    else:
                        nc.scalar.copy(raw[:], acc[:])
                    swp = psp.tile([128, 512], F32, tag="big", bufs=3,
                                   name=f"swp{head_or_k}_{n}")
                    nc.tensor.matmul(swp[:], rmat_sb[:], raw[:],
                                     start=True, stop=True)
                    nc.vector.tensor_mul(dst[:, nsl], swp[:], sin_sb[:, nsl])
                    tmp = wp.tile([128, 512], BF, tag="ropetmp", bufs=2,
                                  name=f"tmp{head_or_k}_{n}")
                    nc.vector.tensor_mul(tmp[:], raw[:], cos_sb[:, nsl])
                    nc.vector.tensor_add(dst[:, nsl], dst[:, nsl], tmp[:])

                def v_drain(n, acc):
                    raw = wp.tile([128, 512], BF, tag="raw", bufs=4,
                                  name=f"rawv_{n}")
                    nc.vector.tensor_copy(raw[:], acc[:])
                    vtr = ps1.tile([128, 512], BF, tag="aux", bufs=2,
                                   name=f"vtr_{n}")
                    for b in range(4):
                        nc.tensor.transpose(vtr[:, b * 128:(b + 1) * 128],
                                            raw[:, b * 128:(b + 1) * 128],
                                            ident_sb[:])
                    nc.scalar.copy(vs[:, n * 512:(n + 1) * 512], vtr[:])

                # ---- pass 1 main loop ----
                for n in range(NQC):
                    acc_k = ps1.tile([128, 512], F32, tag="acc", bufs=5,
                                     name=f"acck_{n}")
                    acc_v = ps1.tile([128, 512], F32, tag="acc", bufs=5,
                                     name=f"accv_{n}")
                    acc_q0 = ps1.tile([128, 512], F32, tag="acc", bufs=5,
                                      name=f"accq0_{n}")
                    acc_q1 = ps1.tile([128, 512], F32, tag="acc", bufs=5,
                                      name=f"accq1_{n}")
                    for g in range(NGRP):
                        xg = wp.tile([128, XG, 512], BF, tag="xg", bufs=4,
                                     name=f"xg1_{n}_{g}")
                        nc.scalar.dma_start(xg[:], xp[:, n, g, :, :])
                        for kk in range(XG):
                            kc = g * XG + kk
                            st, sp = (kc == 0), (kc == NKC - 1)
                            nc.tensor.matmul(acc_k[:], wk_sb[:, kc, :],
                                             xg[:, kk, :], start=st, stop=sp)
                            nc.tensor.matmul(acc_v[:], wv_sb[:, kc, :],
                                             xg[:, kk, :], start=st, stop=sp)
                            nc.tensor.matmul(acc_q0[:], wq01_sb[:, kc, 0:128],
                                             xg[:, kk, :], start=st, stop=sp)
                            nc.tensor.matmul(acc_q1[:], wq01_sb[:, kc, 128:256],
                                             xg[:, kk, :], start=st, stop=sp)
                    rope_drain("k", n, acc_k)
                    rope_drain(0, n, acc_q0, dve_copy=True)
                    rope_drain(1, n, acc_q1)
                    v_drain(n, acc_v)

                ps1.release()

                # ================= pass 2: q1..q3 + attention + out-proj ====
                psp = tc.alloc_tile_pool(name="ps2", bufs=1, space="PSUM")

                def a_head(jj, h):
                    """Attention for chunk jj, head h. The k-block loop is
                    software-pipelined: pv/dn matmuls of block i are emitted
                    after stp/exp of block i+2, so the PE never waits on the
                    DVE/ACT softmax chain. Returns a deferred closure for the
                    final normalization (bc matmul + attn write); the caller
                    emits it after unrelated PE filler so the reciprocal
                    latency is hidden."""
                    jsl = slice(jj * 512, (jj + 1) * 512)
                    nblk = 4 * jj + 4 if mask_mode == "causal" else NKB
                    n_quads = jj if mask_mode == "causal" else NKB // 4
                    dn_total = n_quads + 4 if mask_mode == "causal" else n_quads
                    pv = psp.tile([128, 512], F32, tag="pv", bufs=1,
                                  name=f"pv{h}_{jj}")
                    dn = psp.tile([1, 512], F32, tag="dn", bufs=1,
                                  name=f"dn{h}_{jj}")
                    state = {"pending": None, "ppend": None, "dn_i": 0}
                    fl = []

                    def flush_one():
                        i, pexp, off, diag = fl.pop(0)
                        nc.tensor.matmul(pv[:, off:],
                                         vs[:, i * 128:(i + 1) * 128],
                                         pexp[:, off:],
                                         start=(i == 0),
                                         stop=(i == nblk - 1))
                        dn_i = state["dn_i"]
                        if diag:
                            nc.tensor.matmul(dn[:, off:], ones_sb[:],
                                             pexp[:, off:],
                                             start=(dn_i == 0),
                                             stop=(dn_i == dn_total - 1))
                            state["dn_i"] += 1
                        elif state["pending"] is None:
                            state["pending"] = pexp
                        else:
                            pr = wp.tile([128, 512], BF, tag="ppair", bufs=3,
                                         name=f"pr{h}_{jj}_{i}")
                            nc.vector.tensor_add(pr[:], state["pending"][:],
                                                 pexp[:])
                            state["pending"] = None
                            if state["ppend"] is None:
                                state["ppend"] = pr
                            else:
                                qd = wp.tile([128, 512], BF, tag="quad",
                                             bufs=2, name=f"qd{h}_{jj}_{i}")
                                nc.vector.tensor_add(qd[:], state["ppend"][:],
                                                     pr[:])
                                state["ppend"] = None
                                nc.tensor.matmul(dn[:], ones_sb[:], qd[:],
                                                 start=(dn_i == 0),
                                                 stop=(dn_i == dn_total - 1))
                                state["dn_i"] += 1

                    for i in range(nblk):
                        r = i - 4 * jj
                        diag = mask_mode == "causal" and r >= 0
                        off = 128 * r if (diag and r > 0) else 0
                        stp = psp.tile([128, 512], F32, tag="big", bufs=3,
                                       name=f"st{h}_{jj}_{i}")
                        nc.tensor.matmul(stp[:, off:],
                                         kt[:, i * 128:(i + 1) * 128],
                                         qt[h][:, jj * 512 + off:(jj + 1) * 512],
                                         start=True, stop=True)
                        if mask_mode == "full":
                            mt = wp.tile([128, 512], F32, tag="mt", bufs=3)
                            nc.sync.dma_start(
                                mt[:], maskT_d[i * 128:(i + 1) * 128, jsl])
                            nc.vector.tensor_add(stp[:], stp[:], mt[:])
                        pexp = wp.tile([128, 512], BF, tag="pexp", bufs=6,
                                       name=f"pexp{h}_{jj}_{i}")
                        nc.scalar.activation(pexp[:, off:], stp[:, off:],
                                             EXP, scale=SCALE)
                        if diag:
                            nc.vector.tensor_mul(pexp[:, off:off + 128],
                                                 pexp[:, off:off + 128],
                                                 trimask_sb[:])
                        fl.append((i, pexp, off, diag))
                        if len(fl) > 2:
                            flush_one()
                    while fl:
                        flush_one()
                    assert state["pending"] is None and state["ppend"] is None
                    assert state["dn_i"] == dn_total
                    # reciprocal now (frees the dn bank); the broadcast matmul
                    # and attn write are deferred to hide the DVE latency
                    rcp = wp.tile([1, 512], F32, tag="rcp", bufs=2)
                    nc.vector.reciprocal_approx_fast(rcp[:], dn[:])
                    rcpr = wp.tile([1, 512], BF, tag="rcpr", bufs=2)
                    nc.vector.tensor_copy(rcpr[:], rcp[:])

                    def finalize():
                        bcs = wp.tile([128, 512], BF, tag="bcs", bufs=2)
                        nc.gpsimd.partition_broadcast(bcs[:], rcpr[:])
                        nc.vector.tensor_mul(attn[h][:, jsl], pv[:], bcs[:])
                    return finalize

                def wo_block(m, eng_flip):
                    # one 128-row seq block of the output projection
                    msl = slice(m * 128, (m + 1) * 128)
                    for w4 in range(4):
                        yps = [psp.tile([128, 512], F32, tag="big", bufs=3,
                                        name=f"yp{m}_{w4}_{i}")
                               for i in range(2)]
                        for kc in range(HPC):
                            for i in range(2):
                                ncol = w4 * 2 + i
                                nc.tensor.matmul(
                                    yps[i][:], attn[kc][:, msl],
                                    wo_sb[:, kc, ncol * 512:(ncol + 1) * 512],
                                    start=(kc == 0), stop=(kc == HPC - 1))
                        for i in range(2):
                            ncol = w4 * 2 + i
                            ysb = wp.tile([128, 512], BF, tag="ysb", bufs=6)
                            if (w4 + i + eng_flip) % 2 == 0:
                                nc.scalar.copy(ysb[:], yps[i][:])
                            else:
                                nc.vector.tensor_copy(ysb[:], yps[i][:])
                            nc.sync.dma_start(
                                out[msl, ncol * 512:(ncol + 1) * 512],
                                ysb[:])

                def p2_group(j, g, accs):
                    xg = wp.tile([128, XG, 512], BF, tag="xg", bufs=4,
                                 name=f"xg2_{j}_{g}")
                    nc.scalar.dma_start(xg[:], xp[:, j, g, :, :])
                    for kk in range(XG):
                        kc = g * XG + kk
                        st, sp = (kc == 0), (kc == NKC - 1)
                        for s in range(2):
                            nc.tensor.matmul(
                                accs[s][:],
                                wq23_sb[:, kc, s * 128:(s + 1) * 128],
                                xg[:, kk, :], start=st, stop=sp)

                # ---- pass 2 main loop ----
                deferred = []  # pending attention-normalize closures

                def flush_deferred():
                    while deferred:
                        deferred.pop(0)()

                for j in range(NQC):
                    jj, cc = j - 1, j - 2
                    accs = [psp.tile([128, 512], F32, tag="acc", bufs=3,
                                     name=f"acc2_{j}_{s}")
                            for s in range(2)]
                    for h in range(HPC):
                        for g in range(2 * h, 2 * h + 2):
                            p2_group(j, g, accs)
                        if j == 0 and h == 0:
                            # wo load deferred to here: overlaps pass-2 compute
                            for gg in range(2):
                                nc.sync.dma_start(
                                    wo_sb[:, :, gg * 2048:(gg + 1) * 2048],
                                    wo[:, :, gg * 2048:(gg + 1) * 2048])
                        flush_deferred()
                        if cc >= 0:
                            wo_block(4 * cc + h, h)
                        if jj >= 0:
                            deferred.append(a_head(jj, h))
                    for s, head in enumerate((2, 3)):
                        rope_drain2(head, j, accs[s], psp, dve_copy=(s == 1))

                # ---- tail: attention chunk 3 + out-proj chunks 2,3 ----
                # a_head before wo_block: the out-proj matmuls then stream
                # while the just-flushed normalization chain completes
                for h in range(HPC):
                    flush_deferred()
                    deferred.append(a_head(NQC - 1, h))
                    wo_block(8 + h, h)
                for m in range(12, 16):
                    flush_deferred()
                    wo_block(m, m)
                psp.release()

    nc.compile()
    return nc


def get_program(mask_mode: str):
    if mask_mode not in _PROG_CACHE:
        _PROG_CACHE[mask_mode] = _build_program(mask_mode)
    return _PROG_CACHE[mask_mode]


# ====================== host-side preparation ======================

_PERM128 = np.concatenate([np.arange(0, 128, 2), np.arange(1, 128, 2)])


def _bf16(a: np.ndarray) -> np.ndarray:
    import ml_dtypes
    return np.ascontiguousarray(a.astype(np.float32).astype(ml_dtypes.bfloat16))


def _perm_cols(w: np.ndarray, n_heads: int) -> np.ndarray:
    """Permute each head's 128 columns: even dims first, odd dims last."""
    cols = np.concatenate([h * 128 + _PERM128 for h in range(n_heads)])
    return w[:, cols]


def _classify_mask(mask: np.ndarray) -> str:
    if not np.any(mask):
        return "none"
    iu = np.triu_indices(SEQ, 1)
    upper = mask[iu]
    lower_ok = not np.any(np.tril(mask))
    upper_ok = bool(np.all(np.isneginf(upper) | (upper <= -1e9)))
    if lower_ok and upper_ok:
        return "causal"
    return "full"


def _pack_w(w: np.ndarray) -> np.ndarray:
    """[DIM, M] -> SBUF layout [128, NKC, M] (partition-major, contiguous)."""
    m = w.shape[1]
    return _bf16(w.reshape(NKC, 128, m).transpose(1, 0, 2))


def _host_inputs(x, wq, wk, wv, wo, freqs_cos, freqs_sin, mask):
    wq_p = _perm_cols(np.asarray(wq, np.float32), N_HEADS)
    wk_p = _perm_cols(np.asarray(wk, np.float32), N_KV)
    wv_ = np.asarray(wv, np.float32)
    wo_ = np.asarray(wo, np.float32)

    # x packed to [128, NQC, NGRP, XG, 512]: xp[p,n,g,kk,s] = x[n*512+s,
    # (g*XG+kk)*128+p] — every DMA group is one contiguous multi-KB run.
    x2 = np.asarray(x, np.float32).reshape(SEQ, DIM)
    xpk = _bf16(x2.reshape(NQC, 512, NGRP, XG, 128)
                .transpose(4, 0, 2, 3, 1))

    cosT = np.asarray(freqs_cos, np.float32).T              # [64, SEQ]
    sinT = np.asarray(freqs_sin, np.float32).T
    cos2 = np.concatenate([cosT, cosT], 0)                  # [128, SEQ]
    sin2 = np.concatenate([sinT, sinT], 0)

    rmat = np.zeros((HD, HD), np.float32)
    rmat[np.arange(64) + 64, np.arange(64)] = -1.0   # swp[:64] = -raw[64:]
    rmat[np.arange(64), np.arange(64) + 64] = 1.0    # swp[64:] = raw[:64]
    ident = np.eye(128, dtype=np.float32)

    mask = np.asarray(mask, np.float32)
    mode = _classify_mask(mask)

    common = {"xp": xpk, "cos2": _bf16(cos2), "sin2": _bf16(sin2),
              "rmat": _bf16(rmat), "ident": _bf16(ident),
              "ones_col": _bf16(np.ones((HD, 1), np.float32)),
              "ones_row": _bf16(np.ones((1, HD), np.float32))}
    if mode == "causal":
        kk = np.arange(128)[:, None]
        qq = np.arange(128)[None, :]
        common["trimask"] = _bf16((kk <= qq).astype(np.float32))
    elif mode == "full":
        m = np.where(np.isneginf(mask), NEG, mask)
        common["maskT"] = np.ascontiguousarray(m.T)

    in_maps = []
    for c in range(NCORES):
        im = dict(common)
        wq_c = wq_p[:, c * QD:(c + 1) * QD]
        im["wq01"] = _pack_w(wq_c[:, 0:256])
        im["wq23"] = _pack_w(wq_c[:, 256:512])
        im["wk"] = _pack_w(wk_p[:, c * HD:(c + 1) * HD])
        im["wv"] = _pack_w(wv_[:, c * HD:(c + 1) * HD])
        # wo packed to [128, HPC, DIM]: wo[p,kc,nn] = wo_[kc*128+p, nn]
        im["wo"] = _bf16(wo_[c * QD:(c + 1) * QD, :]
                         .reshape(HPC, 128, DIM).transpose(1, 0, 2))
        in_maps.append(im)
    return mode, in_maps


def _scores_safe(x, wq, wk):
    """The device softmax skips the max-subtraction (scores from
    setup_inputs()-scaled weights are O(5), so exp() is exact and safe).
    Estimate the score magnitude; if exp could overflow fp32, fall back."""
    sx = float(np.sqrt(np.mean(np.square(x), dtype=np.float64)))
    sq = sx * float(np.sqrt(np.mean(np.square(wq), dtype=np.float64)) * np.sqrt(DIM))
    sk = sx * float(np.sqrt(np.mean(np.square(wk), dtype=np.float64)) * np.sqrt(DIM))
    # rope with arbitrary freqs can scale q/k by ~sqrt(2); 7 sigma tail margin
    return 2.0 * sq * sk * 7.0 < 80.0


def _numpy_fallback(x, wq, wk, wv, wo, freqs_cos, freqs_sin, mask):
    """Slow but numerically-safe host path (stable softmax), used only when
    the score magnitudes could overflow the device's unshifted exp."""
    x2 = x.reshape(SEQ, DIM).astype(np.float64)
    q = (x2 @ wq.astype(np.float64)).reshape(SEQ, N_HEADS, HD)
    k = (x2 @ wk.astype(np.float64)).reshape(SEQ, N_KV, HD)
    v = (x2 @ wv.astype(np.float64)).reshape(SEQ, N_KV, HD)
    cos = freqs_cos.astype(np.float64)[:, None, :]
    sin = freqs_sin.astype(np.float64)[:, None, :]

    def rope(t):
        a, b = t[..., 0::2], t[..., 1::2]
        out = np.empty_like(t)
        out[..., 0::2] = a * cos - b * sin
        out[..., 1::2] = a * sin + b * cos
        return out

    q, k = rope(q), rope(k)
    m64 = mask.astype(np.float64)
    outh = np.empty((SEQ, N_HEADS, HD))
    for h in range(N_HEADS):
        g = h // (N_HEADS // N_KV)
        s = q[:, h, :] @ k[:, g, :].T / math.sqrt(HD) + m64
        p = np.exp(s - s.max(-1, keepdims=True))
        p /= p.sum(-1, keepdims=True)
        outh[:, h, :] = p @ v[:, g, :]
    y = outh.reshape(SEQ, N_HEADS * HD) @ wo.astype(np.float64)
    return y.astype(np.float32).reshape(1, SEQ, DIM)


def kernel(x, wq, wk, wv, wo, freqs_cos, freqs_sin, mask, cache_k, cache_v,
           start_pos, **_unused):
    sp = int(np.asarray(start_pos))
    x = np.asarray(x, np.float32)
    wq = np.asarray(wq, np.float32)
    wk = np.asarray(wk, np.float32)
    wv = np.asarray(wv, np.float32)
    wo = np.asarray(wo, np.float32)
    mask = np.asarray(mask, np.float32)
    if sp != 0:
        raise NotImplementedError("kernel assumes start_pos == 0 prefill")
    if not _scores_safe(x, wq, wk):
        return _numpy_fallback(x, wq, wk, wv, wo,
                               np.asarray(freqs_cos, np.float32),
                               np.asarray(freqs_sin, np.float32), mask)

    mode, in_maps = _host_inputs(x, wq, wk, wv, wo,
                                 freqs_cos, freqs_sin, mask)
    nc = get_program(mode)
    res = bass_utils.run_bass_kernel_spmd(nc, in_maps,
                                          core_ids=list(range(NCORES)))
    acc = np.zeros((SEQ, DIM), np.float64)
    for r in res.results:
        acc += r["out"].astype(np.float64)
    return acc.astype(np.float32).reshape(1, SEQ, DIM)
